# revision 1
# baseline (speedup 1.0000x reference)
"""AMNet (BernNet-style GNN) distributed Bass kernel for 8 TRN2 NeuronCores.

Math reformulation (K=2 Bernstein basis):
  reference does 5 sparse props; but with p0 = h, p1 = A_hat h, p2 = A_hat p1:
    B0 = (p0 + 2 p1 + p2)/4,  B1 = (p0 - p2)/2,  B2 = (p0 - 2 p1 + p2)/4
  so only TWO sparse propagations are needed.
  filters: filt_f = sum_k c[f,k] p_k + b_filt[f],  c = relu(theta) @ M
  attention epilogue fully refactored into matmuls (see build()).

Distribution: nodes sharded over 8 cores (12500 each). Edges partitioned by
dst core, bucketed by src core. Each prop: AllGather the (padded fp16) node
table, dma_gather rows per bucket, scale by edge weight (DVE), dma_scatter_add
into the local table.
"""

import math

import numpy as np

import concourse.bass as bass
import concourse.tile as tile
from concourse import bacc, library_config, mybir

FP16 = mybir.dt.float16
F32 = mybir.dt.float32
I16 = mybir.dt.int16
P = 128
AX = mybir.AxisListType
ALU = mybir.AluOpType
ACT = mybir.ActivationFunctionType


def _patch_swdge_lane_assignment():
    """Tile round-robins DMASW sem lanes ignoring queue_num, but each lane is
    locked to one SWDGE queue by the ucode/sim. Pin lane = queue_num + 4*flip
    so multi-queue swdge DMAs get consistent lanes (8 lanes / 4 queues)."""
    import concourse.tile_sem_assignment as tsa
    if getattr(tsa, "_amnet_lane_patch", False):
        return
    tsa._amnet_lane_patch = True
    orig = tsa.TileClockTick._assign_tick

    def _assign_tick(self, inst):
        if (isinstance(inst, tsa.DMAInst)
                and inst.engine == tsa.mybir.EngineType.Pool
                and not isinstance(inst, tsa.bass_isa.UserSyncedRemoteDMADescs)):
            q = getattr(inst, "queue_num", 0) or 0
            flips = getattr(self, "_amnet_qflip", None)
            if flips is None:
                flips = self._amnet_qflip = [0, 0, 0, 0]
            lane = q + 4 * flips[q]
            flips[q] ^= 1
            save = self.next_sw_dma_idx
            self.next_sw_dma_idx = lane
            try:
                return orig(self, inst)
            finally:
                self.next_sw_dma_idx = save
        return orig(self, inst)

    tsa.TileClockTick._assign_tick = _assign_tick


_patch_swdge_lane_assignment()


class Cfg:
    def __init__(self, n, ncore, in_f, hid, out_f, chg):
        assert n % ncore == 0
        self.n = n
        self.ncore = ncore
        self.nloc = n // ncore
        self.in_f = in_f
        self.hid = hid
        self.out_f = out_f
        # padded local rows; >= nloc+1 so the last row can be a scatter trash bin
        self.nloc_pad = ((self.nloc + 1 + P - 1) // P) * P
        self.tiles = self.nloc_pad // P
        self.padw = 256            # fp16 table row elems (512B, 256B-multiple)
        self.chg = chg             # 128-edge groups per scatter chunk (cap)
        self.ch = chg * P          # edges per chunk
        self.gcap = 8              # dma_gather num_idxs cap is 1024 (HW)
        # MLP node-chunk size (PSUM free limit 512)
        self.mlp_chunk = 512


def _wrap16(a, pad_val, total):
    """idx array -> [16, total//16] int16 in the dma_gather wrapped layout."""
    out = np.full(total, pad_val, dtype=np.int16)
    out[: a.shape[0]] = a.astype(np.int16)
    w = out.reshape(total // 16, 16).T  # elem j -> [j%16, j//16]
    return np.tile(w, (8, 1)).copy()   # replicated for the 8 gpsimd cores


def host_preprocess(cfg, x, edge_index, W1, b1, W2, b2, theta, b_filt,
                    Wf, bf, Wx, bx, Wc, bc):
    """Build per-core input maps. Returns (in_maps, B)."""
    n, ncore, nloc = cfg.n, cfg.ncore, cfg.nloc
    nloc_pad, hid, in_f, out_f = cfg.nloc_pad, cfg.hid, cfg.in_f, cfg.out_f

    src = np.asarray(edge_index[0], dtype=np.int64)
    dst = np.asarray(edge_index[1], dtype=np.int64)
    deg = np.bincount(dst, minlength=n).astype(np.float32)
    dinv = (1.0 / np.sqrt(np.maximum(deg, 1.0))).astype(np.float32)
    ewv = dinv[src] * dinv[dst]

    ecore = dst // nloc
    bcore = src // nloc

    # Edges are organized into "rounds": within a (core, bucket), each dst
    # appears at most once per round. dma_scatter_add races on duplicate
    # indices within one instruction, so every scatter chunk must have
    # unique dsts; chunks are sub-ranges of rounds. The chunk plan must be
    # identical across cores (SPMD), so round sizes are maxed over cores.
    CAP_G = cfg.chg  # max 128-edge groups per chunk
    per_core = []    # per core: (es, ed, ev, bco, rank) sorted by (bucket, dst)
    rsizes = {}      # (b, r) -> max size over cores
    for c in range(ncore):
        sel = ecore == c
        es, ed, ev = src[sel], dst[sel], ewv[sel]
        o = np.lexsort((ed, bcore[sel]))
        es, ed, ev = es[o], ed[o], ev[o]
        bco = es // nloc
        key = bco * n + ed
        newrun = np.ones(key.shape[0], dtype=bool)
        newrun[1:] = key[1:] != key[:-1]
        runstart = np.maximum.accumulate(
            np.where(newrun, np.arange(key.shape[0]), 0))
        rank = np.arange(key.shape[0]) - runstart
        per_core.append((es, ed, ev, bco, rank))
        for b in range(ncore):
            m = bco == b
            if not m.any():
                continue
            rk = rank[m]
            cnts = np.bincount(rk)
            for r in range(cnts.shape[0]):
                if cnts[r]:
                    rsizes[(b, r)] = max(rsizes.get((b, r), 0), int(cnts[r]))
    # chunk plan: [(bucket, round, ngroups), ...] — identical across cores
    plan = []
    for b in range(ncore):
        r = 0
        while (b, r) in rsizes:
            glen = (rsizes[(b, r)] + P - 1) // P
            while glen > 0:
                g = min(glen, CAP_G)
                plan.append((b, r, g))
                glen -= g
            r += 1
    B = int(sum(g for (_, _, g) in plan)) * P   # total padded edge slots
    nperm = B // 16

    # ---- weights ----
    thr = np.maximum(np.asarray(theta, np.float64), 0.0)           # relu
    M = np.array([[.25, .5, .25], [.5, 0., -.5], [.25, -.5, .25]], np.float64)
    c3 = (thr @ M)                                                 # [4,3]

    W1 = np.asarray(W1, np.float64); W2 = np.asarray(W2, np.float64)
    Wf = np.asarray(Wf, np.float64); Wx = np.asarray(Wx, np.float64)
    Wc = np.asarray(Wc, np.float64)
    b1 = np.asarray(b1, np.float64); b2 = np.asarray(b2, np.float64)
    bf = np.asarray(bf, np.float64); bx = np.asarray(bx, np.float64)
    bc = np.asarray(bc, np.float64); bflt = np.asarray(b_filt, np.float64)

    w1p = np.concatenate([W1, b1[None, :]], 0).astype(np.float16)      # [in_f+1, hid]
    w2p = np.concatenate([W2, b2[None, :]], 0).astype(np.float16)      # [hid+1, hid]

    wcomb = np.zeros((3 * hid + 1, 4 * hid), np.float64)
    for f in range(4):
        for k in range(3):
            wcomb[k * hid:(k + 1) * hid, f * hid:(f + 1) * hid] = c3[f, k] * Wf
        wcomb[3 * hid, f * hid:(f + 1) * hid] = bflt[f] @ Wf + bf
    wcomb = wcomb.astype(np.float16)

    wxp = np.concatenate([Wx, bx[None, :]], 0).astype(np.float16)      # [hid+1, hid]

    wy = np.zeros((3 * hid, 3 * out_f), np.float64)
    for k in range(3):
        wy[k * hid:(k + 1) * hid, k * out_f:(k + 1) * out_f] = Wc
    wy = wy.astype(np.float16)

    cbm = np.zeros((P, 12), np.float32)          # cols k*4+f = c3[f,k]
    for k in range(3):
        for f in range(4):
            cbm[:, k * 4 + f] = c3[f, k]
    bfc = bflt @ Wc                              # [4, out_f]
    bfcb = np.zeros((P, 4 * out_f), np.float32)  # cols j*4+f = bfc[f,j]+bc[j]
    for j in range(out_f):
        for f in range(4):
            bfcb[:, j * 4 + f] = bfc[f, j] + bc[j]

    tidx = _wrap16(np.arange(cfg.mlp_chunk), 0, cfg.mlp_chunk)

    x = np.asarray(x, np.float32)
    trash = nloc_pad - 1

    in_maps = []
    for c in range(ncore):
        xT = np.zeros((in_f + 1, nloc_pad), np.float16)
        xT[:in_f, :nloc] = x[c * nloc:(c + 1) * nloc].T
        xT[in_f, :] = 1.0

        es, ed, ev, bco, rank = per_core[c]

        # flat (pre-wrap) edge slot arrays; pads: gi=0, si=trash, ew=0.
        # pad slots all target the trash row — scatter races there are
        # harmless (row never read).
        gflat = np.zeros(B, np.int64)
        sflat = np.full(B, trash, np.int64)
        wflat = np.zeros(B, np.float32)
        off = 0
        consumed = {}
        for (b, r, g) in plan:
            m = (bco == b) & (rank == r)
            gs = es[m] - b * nloc
            ss = ed[m] - c * nloc
            vv = ev[m]
            done = consumed.get((b, r), 0)
            take = max(0, min(g * P, gs.shape[0] - done))
            if take > 0:
                gflat[off:off + take] = gs[done:done + take]
                sflat[off:off + take] = ss[done:done + take]
                wflat[off:off + take] = vv[done:done + take]
            consumed[(b, r)] = done + take
            off += g * P
        assert off == B

        gi = np.tile(gflat.astype(np.int16).reshape(B // 16, 16).T,
                     (8, 1)).copy()
        si = np.tile(sflat.astype(np.int16).reshape(B // 16, 16).T,
                     (8, 1)).copy()
        ewt = wflat.astype(np.float16).reshape(B // P, P).T.copy()

        in_maps.append({
            "xT": xT,
            "w1": w1p, "w2": w2p, "wcomb": wcomb, "wxp": wxp, "wy": wy,
            "cb": cbm, "bfcb": bfcb,
            "gi": gi, "si": si, "ew": ewt, "tidx": tidx,
            "ztab": np.zeros((nloc_pad, cfg.padw), np.float16),
        })
    return in_maps, B, plan


def build(cfg, B, plan):
    """Build the SPMD Bass graph. All cores run this same program."""
    ncore, nloc_pad, hid, in_f, out_f, padw = (
        cfg.ncore, cfg.nloc_pad, cfg.hid, cfg.in_f, cfg.out_f, cfg.padw)
    nperm = B // 16
    rg = [list(range(ncore))]
    h2 = hid - P            # 28
    MC = cfg.mlp_chunk

    nc = bacc.Bacc(None, num_devices=ncore, num_swdge_queues=4)

    dp = nc.declare_dram_parameter
    xT_d = dp("xT", [in_f + 1, nloc_pad], FP16, isOutput=False)
    w1_d = dp("w1", [in_f + 1, hid], FP16, isOutput=False)
    w2_d = dp("w2", [hid + 1, hid], FP16, isOutput=False)
    wcomb_d = dp("wcomb", [3 * hid + 1, 4 * hid], FP16, isOutput=False)
    wxp_d = dp("wxp", [hid + 1, hid], FP16, isOutput=False)
    wy_d = dp("wy", [3 * hid, 3 * out_f], FP16, isOutput=False)
    cb_d = dp("cb", [P, 12], F32, isOutput=False)
    bfcb_d = dp("bfcb", [P, 4 * out_f], F32, isOutput=False)
    gi_d = dp("gi", [P, nperm], I16, isOutput=False)
    si_d = dp("si", [P, nperm], I16, isOutput=False)
    ew_d = dp("ew", [P, B // P], FP16, isOutput=False)
    tidx_d = dp("tidx", [P, MC // 16], I16, isOutput=False)
    ztab_d = dp("ztab", [nloc_pad, padw], FP16, isOutput=False)
    out_d = dp("out", [nloc_pad, out_f], F32, isOutput=True)

    h_tab = nc.dram_tensor("h_tab", [nloc_pad, padw], FP16)
    p1_tab = nc.dram_tensor("p1_tab", [nloc_pad, padw], FP16)
    p2_tab = nc.dram_tensor("p2_tab", [nloc_pad, padw], FP16)
    NT = 4
    p1_acc = [nc.dram_tensor(f"p1_acc{i}", [nloc_pad, padw], FP16)
              for i in range(NT)]
    p2_acc = [nc.dram_tensor(f"p2_acc{i}", [nloc_pad, padw], FP16)
              for i in range(NT)]
    h_full = nc.dram_tensor("h_full", [ncore * nloc_pad, padw], FP16,
                            addr_space="Shared")
    p1_full = nc.dram_tensor("p1_full", [ncore * nloc_pad, padw], FP16,
                             addr_space="Shared")

    with tile.TileContext(nc, num_cores=ncore) as tc:
        with tc.tile_pool(name="res", bufs=1) as res:
            # ---------------- resident loads ----------------
            def load(pool, dram, shape, dt, name):
                t = pool.tile(shape, dt, name=name, tag=name)
                nc.sync.dma_start(out=t[:], in_=dram[:])
                return t

            w1a = load(res, w1_d[0:P, :], [P, hid], FP16, "w1a")
            w1b = load(res, w1_d[P:in_f + 1, :], [in_f + 1 - P, hid], FP16, "w1b")
            w2a = load(res, w2_d[0:P, :], [P, hid], FP16, "w2a")
            w2b = load(res, w2_d[P:hid + 1, :], [hid + 1 - P, hid], FP16, "w2b")
            gi_s = load(res, gi_d[:, :], [P, nperm], I16, "gi_s")
            si_s = load(res, si_d[:, :], [P, nperm], I16, "si_s")
            ew_s = load(res, ew_d[:, :], [P, B // P], FP16, "ew_s")
            tidx_s = load(res, tidx_d[:, :], [P, MC // 16], I16, "tidx_s")
            wca = [load(res, wcomb_d[k * hid:k * hid + P, :], [P, 4 * hid], FP16,
                        f"wca{k}") for k in range(3)]
            wcb = [load(res, wcomb_d[k * hid + P:(k + 1) * hid, :], [h2, 4 * hid],
                        FP16, f"wcb{k}") for k in range(3)]
            wcbias = load(res, wcomb_d[3 * hid:3 * hid + 1, :], [1, 4 * hid], FP16,
                          "wcbias")
            wxa = load(res, wxp_d[0:P, :], [P, hid], FP16, "wxa")
            wxb = load(res, wxp_d[P:hid, :], [h2, hid], FP16, "wxb")
            wxbias = load(res, wxp_d[hid:hid + 1, :], [1, hid], FP16, "wxbias")
            wya = [load(res, wy_d[k * hid:k * hid + P, :], [P, 3 * out_f], FP16,
                        f"wya{k}") for k in range(3)]
            wyb = [load(res, wy_d[k * hid + P:(k + 1) * hid, :], [h2, 3 * out_f],
                        FP16, f"wyb{k}") for k in range(3)]
            cb_s = load(res, cb_d, [P, 12], F32, "cb_s")
            bfcb_s = load(res, bfcb_d, [P, 4 * out_f], F32, "bfcb_s")

            ones1 = res.tile([1, P], FP16)
            nc.vector.memset(ones1[:], 1.0)
            outbuf = res.tile([P, cfg.tiles * out_f], F32)

            # zero-init scatter accumulators (pad cols of h_tab too)
            nc.sync.dma_start(out=h_tab[:], in_=ztab_d[:])
            for t in p1_acc + p2_acc:
                nc.sync.dma_start(out=t[:], in_=ztab_d[:])

            # ---------------- MLP: h = relu(x@W1+b1)@W2+b2 ----------------
            with (
                tc.tile_pool(name="mlp", bufs=3) as mlp,
                tc.tile_pool(name="psm", bufs=2, space="PSUM") as psA,
                tc.tile_pool(name="psm2", bufs=2, space="PSUM") as psB,
            ):
              for m0 in range(0, nloc_pad, MC):
                  F = min(MC, nloc_pad - m0)
                  xa = mlp.tile([P, F], FP16, tag="xa")
                  xb = mlp.tile([in_f + 1 - P, F], FP16, tag="xb")
                  nc.sync.dma_start(out=xa[:], in_=xT_d[0:P, m0:m0 + F])
                  nc.sync.dma_start(out=xb[:], in_=xT_d[P:in_f + 1, m0:m0 + F])
                  pa = psA.tile([P, F], F32, tag="mlp_pa")
                  pb = psA.tile([h2, F], F32, tag="mlp_pb")
                  nc.tensor.matmul(pa[:], w1a[:, 0:P], xa[:], start=True, stop=False)
                  nc.tensor.matmul(pa[:], w1b[:, 0:P], xb[:], start=False, stop=True)
                  nc.tensor.matmul(pb[:], w1a[:, P:hid], xa[:], start=True, stop=False)
                  nc.tensor.matmul(pb[:], w1b[:, P:hid], xb[:], start=False, stop=True)
                  t1a = mlp.tile([P, F], FP16, tag="t1a")
                  t1b = mlp.tile([hid + 1 - P, F], FP16, tag="t1b")
                  nc.scalar.activation(t1a[:], pa[:], ACT.Relu)
                  nc.vector.memset(t1b[:], 1.0)
                  nc.scalar.activation(t1b[0:h2, :], pb[:], ACT.Relu)
                  for s0 in range(0, F, P):
                      hp = psB.tile([P, hid], F32, tag="mlp_hp")
                      nc.tensor.matmul(hp[:], t1a[:, s0:s0 + P], w2a[:],
                                       start=True, stop=False)
                      nc.tensor.matmul(hp[:], t1b[:, s0:s0 + P], w2b[:],
                                       start=False, stop=True)
                      hrow = mlp.tile([P, hid], FP16, tag="hrow")
                      nc.scalar.activation(hrow[:], hp[:], ACT.Copy)
                      nc.sync.dma_start(
                          out=h_tab[m0 + s0:m0 + s0 + P, 0:hid], in_=hrow[:])

            # ---------------- propagation ----------------
            def prop(gat, src_tab_full, dst_accs, dst_tab):
                off = 0
                qn = 0
                for ci, (b, r, ng) in enumerate(plan):
                    ch = ng * P
                    tslice = src_tab_full[b * nloc_pad:(b + 1) * nloc_pad, :]
                    i16 = off // 16
                    ie = off // P
                    g = gat.tile([P, ng, padw], FP16, tag="g", name="g",
                                 padded_shape=[P, cfg.chg, padw])
                    # dma_gather is limited to 1024 indices per instruction
                    for go in range(0, ng, cfg.gcap):
                        gn = min(cfg.gcap, ng - go)
                        nc.gpsimd.dma_gather(
                            g[:, go:go + gn, :], tslice,
                            gi_s[:, i16 + go * 8:i16 + (go + gn) * 8],
                            gn * P, gn * P, padw, queue_num=qn % 4)
                        qn += 1
                    ewsl = ew_s[:, ie:ie + ng]
                    nc.vector.tensor_tensor(
                        out=g[:, :, 0:hid], in0=g[:, :, 0:hid],
                        in1=ewsl.to_broadcast((P, ng, hid)),
                        op=ALU.mult)
                    nc.gpsimd.dma_scatter_add(
                        dst_accs[ci % len(dst_accs)][:], g[:],
                        si_s[:, i16:i16 + ch // 16],
                        ch, ch, padw, queue_num=ci % 4)
                    off += ch
                # merge the split accumulators into dst_tab
                for t0 in range(0, nloc_pad, 8 * P):
                    rows = min(8 * P, nloc_pad - t0)
                    w = (rows // P) * padw
                    seg = slice(t0, t0 + rows)
                    view = lambda d: d[seg, :].rearrange(
                        "(a b) w -> a (b w)", a=P)
                    ma = gat.tile([P, w], FP16, tag="ma", name="ma",
                                  padded_shape=[P, 8 * padw], bufs=2)
                    mb = gat.tile([P, w], FP16, tag="mb", name="mb",
                                  padded_shape=[P, 8 * padw], bufs=4)
                    nc.sync.dma_start(out=ma[:], in_=view(dst_accs[0]))
                    eng = [nc.scalar, nc.sync]
                    for ai in range(1, len(dst_accs)):
                        eng[ai % 2].dma_start(out=mb[:], in_=view(dst_accs[ai]))
                        nc.vector.tensor_add(ma[:], ma[:], mb[:])
                    nc.sync.dma_start(out=view(dst_tab), in_=ma[:])

            with tc.tile_pool(name="gat", bufs=6) as gat:
                nc.gpsimd.collective_compute(
                    "AllGather", ALU.bypass, replica_groups=rg,
                    ins=[h_tab[:]], outs=[h_full[:]])
                prop(gat, h_full, p1_acc, p1_tab)
                nc.gpsimd.collective_compute(
                    "AllGather", ALU.bypass, replica_groups=rg,
                    ins=[p1_tab[:]], outs=[p1_full[:]])
                prop(gat, p1_full, p2_acc, p2_tab)

            # ---------------- epilogue ----------------
            with (
                tc.tile_pool(name="epi", bufs=2) as epi,
                tc.tile_pool(name="pse", bufs=2, space="PSUM") as psA,
                tc.tile_pool(name="pse2", bufs=2, space="PSUM") as psB,
            ):
              for m0 in range(0, nloc_pad, MC):
                  F = min(MC, nloc_pad - m0)
                  tT = []
                  for ti, tab in enumerate((h_tab, p1_tab, p2_tab)):
                      t = epi.tile([P, padw // P, F], FP16, tag=f"tT{ti}",
                                   name=f"tT{ti}")
                      nc.gpsimd.dma_gather(
                          t[:], tab[m0:m0 + F, :],
                          tidx_s[:, 0:F // 16], F, F, padw, transpose=True,
                          queue_num=ti % 4)
                      tT.append(t)
                  for s0 in range(0, F, P):
                      sl = slice(s0, s0 + P)
                      pc = [psA.tile([P, 2 * hid], F32, tag=f"comb{i}",
                                     name=f"comb{i}") for i in range(2)]
                      for i in range(2):
                          cs = slice(i * 2 * hid, (i + 1) * 2 * hid)
                          for k in range(3):
                              nc.tensor.matmul(pc[i][:], tT[k][:, 0, sl],
                                               wca[k][:, cs],
                                               start=(k == 0), stop=False)
                              nc.tensor.matmul(pc[i][:], tT[k][0:h2, 1, sl],
                                               wcb[k][:, cs],
                                               start=False, stop=False)
                          nc.tensor.matmul(pc[i][:], ones1[:], wcbias[:, cs],
                                           start=False, stop=True)
                      px = psB.tile([P, hid], F32, tag="px")
                      nc.tensor.matmul(px[:], tT[0][:, 0, sl], wxa[:],
                                       start=True, stop=False)
                      nc.tensor.matmul(px[:], tT[0][0:h2, 1, sl], wxb[:],
                                       start=False, stop=False)
                      nc.tensor.matmul(px[:], ones1[:], wxbias[:],
                                       start=False, stop=True)
                      py = psB.tile([P, 3 * out_f], F32, tag="py")
                      for k in range(3):
                          nc.tensor.matmul(py[:], tT[k][:, 0, sl], wya[k][:],
                                           start=(k == 0), stop=False)
                          nc.tensor.matmul(py[:], tT[k][0:h2, 1, sl], wyb[k][:],
                                           start=False, stop=(k == 2))
                      hp0 = epi.tile([P, 2 * hid], FP16, tag="hp0")
                      hp1 = epi.tile([P, 2 * hid], FP16, tag="hp1")
                      xp = epi.tile([P, hid], FP16, tag="xp")
                      nc.scalar.activation(hp0[:], pc[0][:], ACT.Tanh)
                      nc.scalar.activation(hp1[:], pc[1][:], ACT.Tanh)
                      nc.scalar.activation(xp[:], px[:], ACT.Tanh)
                      scr = epi.tile([P, hid], F32, tag="scr")
                      logit = epi.tile([P, 4], F32, tag="logit")
                      for f in range(4):
                          hsrc = (hp0, hp1)[f // 2]
                          nc.vector.tensor_mul(
                              scr[:],
                              hsrc[:, (f % 2) * hid:(f % 2 + 1) * hid], xp[:])
                          nc.vector.tensor_reduce(
                              logit[:, f:f + 1], scr[:], AX.X, ALU.add)
                      mxn = epi.tile([P, 1], F32, tag="mxn")
                      nc.vector.tensor_reduce(mxn[:], logit[:], AX.X, ALU.max,
                                              negate=True)
                      el = epi.tile([P, 4], F32, tag="el")
                      nc.scalar.activation(el[:], logit[:], ACT.Exp, bias=mxn[:, 0:1])
                      sm = epi.tile([P, 1], F32, tag="sm")
                      nc.vector.tensor_reduce(sm[:], el[:], AX.X, ALU.add)
                      rs = epi.tile([P, 1], F32, tag="rs")
                      nc.vector.reciprocal(rs[:], sm[:])
                      score = epi.tile([P, 4], F32, tag="score")
                      nc.vector.tensor_scalar_mul(score[:], el[:], rs[:, 0:1])
                      scr4 = epi.tile([P, 4], F32, tag="scr4")
                      wk = epi.tile([P, 3], F32, tag="wk")
                      sbf = epi.tile([P, out_f], F32, tag="sbf")
                      for k in range(3):
                          nc.vector.tensor_mul(scr4[:], score[:],
                                               cb_s[:, k * 4:(k + 1) * 4])
                          nc.vector.tensor_reduce(wk[:, k:k + 1], scr4[:],
                                                  AX.X, ALU.add)
                      for j in range(out_f):
                          nc.vector.tensor_mul(scr4[:], score[:],
                                               bfcb_s[:, j * 4:(j + 1) * 4])
                          nc.vector.tensor_reduce(sbf[:, j:j + 1], scr4[:],
                                                  AX.X, ALU.add)
                      tgl = (m0 + s0) // P
                      ob = outbuf[:, tgl * out_f:(tgl + 1) * out_f]
                      scr2 = epi.tile([P, out_f], F32, tag="scr2")
                      nc.vector.tensor_scalar_mul(ob, py[:, 0:out_f], wk[:, 0:1])
                      nc.vector.tensor_scalar_mul(scr2[:], py[:, out_f:2 * out_f],
                                                  wk[:, 1:2])
                      nc.vector.tensor_add(ob, ob, scr2[:])
                      nc.vector.tensor_scalar_mul(scr2[:], py[:, 2 * out_f:3 * out_f],
                                                  wk[:, 2:3])
                      nc.vector.tensor_add(ob, ob, scr2[:])
                      nc.vector.tensor_add(ob, ob, sbf[:])

            nc.sync.dma_start(
                out=out_d[:].rearrange("(t p) j -> p t j", p=P),
                in_=outbuf[:].rearrange("p (t j) -> p t j", j=out_f))
    nc.compile()
    return nc


def numpy_model(cfg, in_maps, B, plan):
    """Bit-approximate numpy model of what the device computes (f32 math),
    for validating the kernel structure without hardware."""
    ncore, nloc_pad, hid, out_f = cfg.ncore, cfg.nloc_pad, cfg.hid, cfg.out_f
    nperm = B // 16
    outs = []
    # build tables per core
    h_tabs = []
    for c in range(ncore):
        im = in_maps[c]
        xT = im["xT"].astype(np.float32)
        w1 = im["w1"].astype(np.float32)
        w2 = im["w2"].astype(np.float32)
        t1 = np.maximum(xT.T @ w1, 0.0)
        t1 = np.concatenate([t1, np.ones((nloc_pad, 1), np.float32)], 1)
        h = t1 @ w2
        tab = np.zeros((nloc_pad, cfg.padw), np.float32)
        tab[:, :hid] = h
        h_tabs.append(tab)

    def prop_all(tabs):
        full = np.concatenate(tabs, 0)
        res = []
        for c in range(ncore):
            im = in_maps[c]
            acc = np.zeros((nloc_pad, cfg.padw), np.float32)
            gi_f = im["gi"][:16, :].T.reshape(-1)
            si_f = im["si"][:16, :].T.reshape(-1)
            ew_f = im["ew"].T.reshape(-1).astype(np.float32)
            off = 0
            for (b, r, g) in plan:
                ch = g * P
                gi = gi_f[off:off + ch]
                si = si_f[off:off + ch]
                ewf = ew_f[off:off + ch]
                rows = full[b * nloc_pad + gi.astype(np.int64), :].astype(
                    np.float16).astype(np.float32)
                rows[:, :hid] *= ewf[:, None]
                np.add.at(acc, si.astype(np.int64), rows)
                off += ch
            res.append(acc.astype(np.float16).astype(np.float32))
        return res

    p1_tabs = prop_all(h_tabs)
    p2_tabs = prop_all(p1_tabs)

    for c in range(ncore):
        im = in_maps[c]
        hT = h_tabs[c][:, :hid]
        p1 = p1_tabs[c][:, :hid]
        p2 = p2_tabs[c][:, :hid]
        stack = np.concatenate([hT, p1, p2, np.ones((nloc_pad, 1), np.float32)], 1)
        comb = stack @ im["wcomb"].astype(np.float32)
        hproj = np.tanh(comb).reshape(nloc_pad, 4, hid)
        xp = np.tanh(np.concatenate([hT, np.ones((nloc_pad, 1), np.float32)], 1)
                     @ im["wxp"].astype(np.float32))
        logits = np.einsum("nfd,nd->nf", hproj, xp)
        e = np.exp(logits - logits.max(1, keepdims=True))
        score = e / e.sum(1, keepdims=True)
        y = (stack[:, :3 * hid] @ im["wy"].astype(np.float32)).reshape(
            nloc_pad, 3, out_f)
        wk = np.stack([ (score * im["cb"][0, k * 4:(k + 1) * 4][None, :]).sum(1)
                        for k in range(3)], 1)
        sbf = np.stack([(score * im["bfcb"][0, j * 4:(j + 1) * 4][None, :]).sum(1)
                        for j in range(out_f)], 1)
        out = (y * wk[:, :, None]).sum(1) + sbf
        outs.append(out.astype(np.float32))
    return outs


# ---------------------------------------------------------------------------
# Self-contained harness entry point: kernel(**inputs) -> np.ndarray
# ---------------------------------------------------------------------------
_NC_CACHE = {}


def kernel(**inputs):
    """AMNet forward on 8 TRN2 NeuronCores. Takes full unsharded inputs,
    returns the full [N, 2] float32 output."""
    from concourse.bass_utils import run_bass_kernel_spmd

    cfg = Cfg(n=100000, ncore=8, in_f=166, hid=156, out_f=2, chg=16)
    in_maps, B, plan = host_preprocess(cfg, **inputs)
    key = (B, tuple(plan))
    nc = _NC_CACHE.get(key)
    if nc is None:
        nc = build(cfg, B, plan)
        _NC_CACHE[key] = nc
    res = run_bass_kernel_spmd(nc, in_maps,
                               core_ids=list(range(cfg.ncore)), trace=False)
    out = np.concatenate(
        [res.results[i]["out"][:cfg.nloc] for i in range(cfg.ncore)], 0)
    return out.astype(np.float32)



# revision 8
# speedup vs baseline: 1.7190x; 1.7190x over previous
"""AMNet (BernNet-style GNN) distributed Bass kernel for 8 TRN2 NeuronCores.

Math reformulation (K=2 Bernstein basis):
  reference does 5 sparse props; but with p0 = h, p1 = A_hat h, p2 = A_hat p1:
    B0 = (p0 + 2 p1 + p2)/4,  B1 = (p0 - p2)/2,  B2 = (p0 - 2 p1 + p2)/4
  so only TWO sparse propagations are needed.
  filters: filt_f = sum_k c[f,k] p_k + b_filt[f],  c = relu(theta) @ M
  attention epilogue fully refactored into matmuls (see build()).

Distribution: nodes sharded over 8 cores (12500 each). Edges partitioned by
dst core. Each prop: AllGather the (padded fp16) node table to h_full, then
per 128-dst block: dma_gather the source rows (sorted by dst block), build a
weighted one-hot selector on DVE (sel[e,d] = ew[e] * (dstoff[e]==d)) and
segment-reduce on TensorE into a PSUM tile — NO dma_scatter_add, no
accumulator tables, no merge phase. Block results stream out sequentially.
"""

import math

import numpy as np

import concourse.bass as bass
import concourse.tile as tile
from concourse import bacc, library_config, mybir

FP16 = mybir.dt.float16
F32 = mybir.dt.float32
I16 = mybir.dt.int16
P = 128
AX = mybir.AxisListType
ALU = mybir.AluOpType
ACT = mybir.ActivationFunctionType

QROWS = 32768          # gather source slice rows (int16 index range)


def _patch_swdge_lane_assignment():
    """Tile round-robins DMASW sem lanes ignoring queue_num, but each lane is
    locked to one SWDGE queue by the ucode/sim. Pin lane = queue_num + 4*flip
    so multi-queue swdge DMAs get consistent lanes (8 lanes / 4 queues)."""
    import concourse.tile_sem_assignment as tsa
    if getattr(tsa, "_amnet_lane_patch", False):
        return
    tsa._amnet_lane_patch = True
    orig = tsa.TileClockTick._assign_tick

    def _assign_tick(self, inst):
        if (isinstance(inst, tsa.DMAInst)
                and inst.engine == tsa.mybir.EngineType.Pool
                and not isinstance(inst, tsa.bass_isa.UserSyncedRemoteDMADescs)):
            q = getattr(inst, "queue_num", 0) or 0
            flips = getattr(self, "_amnet_qflip", None)
            if flips is None:
                flips = self._amnet_qflip = [0, 0, 0, 0]
            lane = q + 4 * flips[q]
            flips[q] ^= 1
            save = self.next_sw_dma_idx
            self.next_sw_dma_idx = lane
            try:
                return orig(self, inst)
            finally:
                self.next_sw_dma_idx = save
        return orig(self, inst)

    tsa.TileClockTick._assign_tick = _assign_tick


_patch_swdge_lane_assignment()


class Cfg:
    def __init__(self, n, ncore, in_f, hid, out_f):
        assert n % ncore == 0
        self.n = n
        self.ncore = ncore
        self.nloc = n // ncore
        self.in_f = in_f
        self.hid = hid
        self.out_f = out_f
        self.nloc_pad = ((self.nloc + P - 1) // P) * P
        self.tiles = self.nloc_pad // P       # dst blocks per core
        self.padw = 256            # fp16 table row elems (512B, 256B-multiple)
        self.gcap = 8              # dma_gather num_idxs cap is 1024 (HW)
        self.gbufs = 8             # gather buffer ring size
        # MLP node-chunk size (PSUM free limit 512)
        self.mlp_chunk = 512


def _wrap16(a, pad_val, total):
    """idx array -> [128, total//16] int16 in the dma_gather wrapped layout."""
    out = np.full(total, pad_val, dtype=np.int16)
    out[: a.shape[0]] = a.astype(np.int16)
    w = out.reshape(total // 16, 16).T  # elem j -> [j%16, j//16]
    return np.tile(w, (8, 1)).copy()   # replicated for the 8 gpsimd cores


def host_preprocess(cfg, x, edge_index, W1, b1, W2, b2, theta, b_filt,
                    Wf, bf, Wx, bx, Wc, bc):
    """Build per-core input maps. Returns (in_maps, G_total, plan)."""
    n, ncore, nloc = cfg.n, cfg.ncore, cfg.nloc
    nloc_pad, hid, in_f, out_f = cfg.nloc_pad, cfg.hid, cfg.in_f, cfg.out_f

    src = np.asarray(edge_index[0], dtype=np.int64)
    dst = np.asarray(edge_index[1], dtype=np.int64)
    deg = np.bincount(dst, minlength=n).astype(np.float32)
    dinv = (1.0 / np.sqrt(np.maximum(deg, 1.0))).astype(np.float32)
    ewv = dinv[src] * dinv[dst]

    # global row in the concatenated (padded) full table
    src_row = (src // nloc) * nloc_pad + (src % nloc)
    nq = (ncore * nloc_pad + QROWS - 1) // QROWS

    ecore = dst // nloc

    # Per core: sort edges by (dst block, src slice q, src_row).
    per_core = []
    cnts = np.zeros((ncore, cfg.tiles, nq), np.int64)
    for c in range(ncore):
        sel = ecore == c
        es_row, ed, ev = src_row[sel], dst[sel] - c * nloc, ewv[sel]
        blk = ed // P
        q = es_row // QROWS
        o = np.lexsort((es_row, q, blk))
        es_row, ed, ev, blk, q = es_row[o], ed[o], ev[o], blk[o], q[o]
        per_core.append((es_row, ed, ev))
        np.add.at(cnts[c], (blk, q), 1)

    # plan: [(block, q, ngroups, nvalid)] identical across cores; per-entry
    # slot counts are maxed over cores and rounded up to whole 128-groups.
    # nvalid = number of non-skipped descriptors in the call (the gather
    # ucode requires num_idxs_reg == count of non-negative indices, so every
    # core pads its real edges with idx=0 descriptors up to nvalid).
    maxc = cnts.max(axis=0)  # [tiles, nq]
    plan = []
    for b in range(cfg.tiles):
        for q in range(nq):
            mc = int(maxc[b, q])
            g = (mc + P - 1) // P
            off = 0
            while g > 0:
                take = min(g, cfg.gcap)
                nvalid = min(mc - off, take * P)
                plan.append((b, q, take, nvalid))
                off += take * P
                g -= take
    G_total = sum(g for (_, _, g, _) in plan)
    B = G_total * P

    # ---- weights ----
    h2 = hid - P
    thr = np.maximum(np.asarray(theta, np.float64), 0.0)           # relu
    M = np.array([[.25, .5, .25], [.5, 0., -.5], [.25, -.5, .25]], np.float64)
    c3 = (thr @ M)                                                 # [4,3]

    W1 = np.asarray(W1, np.float64); W2 = np.asarray(W2, np.float64)
    Wf = np.asarray(Wf, np.float64); Wx = np.asarray(Wx, np.float64)
    Wc = np.asarray(Wc, np.float64)
    b1 = np.asarray(b1, np.float64); b2 = np.asarray(b2, np.float64)
    bf = np.asarray(bf, np.float64); bx = np.asarray(bx, np.float64)
    bc = np.asarray(bc, np.float64); bflt = np.asarray(b_filt, np.float64)

    w1p = np.concatenate([W1, b1[None, :]], 0).astype(np.float16)      # [in_f+1, hid]
    w2p = np.concatenate([W2, b2[None, :]], 0).astype(np.float16)      # [hid+1, hid]

    wcomb = np.zeros((3 * hid + 1, 4 * hid), np.float64)
    for f in range(4):
        for k in range(3):
            wcomb[k * hid:(k + 1) * hid, f * hid:(f + 1) * hid] = c3[f, k] * Wf
        wcomb[3 * hid, f * hid:(f + 1) * hid] = bflt[f] @ Wf + bf
    wcomb = wcomb.astype(np.float16)

    wxp = np.concatenate([Wx, bx[None, :]], 0).astype(np.float16)      # [hid+1, hid]

    wy = np.zeros((3 * hid, 3 * out_f), np.float64)
    for k in range(3):
        wy[k * hid:(k + 1) * hid, k * out_f:(k + 1) * out_f] = Wc
    wy = wy.astype(np.float16)

    cbm = np.zeros((P, 12), np.float32)          # cols k*4+f = c3[f,k]
    for k in range(3):
        for f in range(4):
            cbm[:, k * 4 + f] = c3[f, k]
    bfc = bflt @ Wc                              # [4, out_f]
    bfcb = np.zeros((P, 4 * out_f), np.float32)  # cols j*4+f = bfc[f,j]+bc[j]
    for j in range(out_f):
        for f in range(4):
            bfcb[:, j * 4 + f] = bfc[f, j] + bc[j]

    tidx = _wrap16(np.arange(cfg.mlp_chunk), 0, cfg.mlp_chunk)

    x = np.asarray(x, np.float32)

    in_maps = []
    for c in range(ncore):
        xT = np.zeros((in_f + 1, nloc_pad), np.float16)
        xT[:in_f, :nloc] = x[c * nloc:(c + 1) * nloc].T
        xT[in_f, :] = 1.0

        es_row, ed, ev = per_core[c]
        blk = ed // P
        q = es_row // QROWS
        # per (b, q) run boundaries in the sorted arrays
        # fill flat slot arrays per plan entry
        gflat = np.full(B, -1, np.int64)       # -1 = skipped descriptor
        doflat = np.full(B, -1.0, np.float32)  # -1 = sel matches nothing
        ewflat = np.zeros(B, np.float32)
        key = blk * nq + q
        starts = np.searchsorted(key, np.arange(cfg.tiles * nq), side="left")
        ends = np.searchsorted(key, np.arange(cfg.tiles * nq), side="right")
        consumed = {}
        off = 0
        for (b, qq, g, nvalid) in plan:
            kidx = b * nq + qq
            s0, s1 = int(starts[kidx]), int(ends[kidx])
            done = consumed.get(kidx, 0)
            take = max(0, min(g * P, (s1 - s0) - done))
            if take > 0:
                sl = slice(s0 + done, s0 + done + take)
                gflat[off:off + take] = es_row[sl] - qq * QROWS
                doflat[off:off + take] = (ed[sl] - b * P).astype(np.float32)
                ewflat[off:off + take] = ev[sl]
            # pad with idx=0 descriptors (sel-zeroed) up to the call's
            # shared valid count; the rest stay -1 (skipped)
            if nvalid > take:
                gflat[off + take:off + nvalid] = 0
            consumed[kidx] = done + take
            off += g * P
        assert off == B

        gi = np.tile(gflat.astype(np.int16).reshape(B // 16, 16).T,
                     (8, 1)).copy()
        # slot j of group g -> partition j%128; DVE scalar tables are
        # [P, G_total] with column g holding slots [g*128, (g+1)*128)
        dot = doflat.reshape(G_total, P).T.copy()
        ewt = ewflat.reshape(G_total, P).T.copy()

        in_maps.append({
            "xT": xT,
            "w1": w1p, "w2": w2p, "wcomb": wcomb, "wxp": wxp, "wy": wy,
            "cb": cbm, "bfcb": bfcb,
            "gi": gi, "do": dot, "ew": ewt, "tidx": tidx,
        })
    return in_maps, G_total, plan


def build(cfg, G_total, plan):
    """Build the SPMD Bass graph. All cores run this same program."""
    ncore, nloc_pad, hid, in_f, out_f, padw = (
        cfg.ncore, cfg.nloc_pad, cfg.hid, cfg.in_f, cfg.out_f, cfg.padw)
    B = G_total * P
    nperm = B // 16
    rg = [list(range(ncore))]
    h2 = hid - P            # 28
    MC = cfg.mlp_chunk
    nq = (ncore * nloc_pad + QROWS - 1) // QROWS

    nc = bacc.Bacc(None, num_devices=ncore, num_swdge_queues=4)

    dp = nc.declare_dram_parameter
    xT_d = dp("xT", [in_f + 1, nloc_pad], FP16, isOutput=False)
    w1_d = dp("w1", [in_f + 1, hid], FP16, isOutput=False)
    w2_d = dp("w2", [hid + 1, hid], FP16, isOutput=False)
    wcomb_d = dp("wcomb", [3 * hid + 1, 4 * hid], FP16, isOutput=False)
    wxp_d = dp("wxp", [hid + 1, hid], FP16, isOutput=False)
    wy_d = dp("wy", [3 * hid, 3 * out_f], FP16, isOutput=False)
    cb_d = dp("cb", [P, 12], F32, isOutput=False)
    bfcb_d = dp("bfcb", [P, 4 * out_f], F32, isOutput=False)
    gi_d = dp("gi", [P, nperm], I16, isOutput=False)
    do_d = dp("do", [P, G_total], F32, isOutput=False)
    ew_d = dp("ew", [P, G_total], F32, isOutput=False)
    tidx_d = dp("tidx", [P, MC // 16], I16, isOutput=False)
    out_d = dp("out", [nloc_pad, out_f], F32, isOutput=True)

    h_tab = nc.dram_tensor("h_tab", [nloc_pad, padw], FP16)
    p1_tab = nc.dram_tensor("p1_tab", [nloc_pad, padw], FP16)
    p2_tab = nc.dram_tensor("p2_tab", [nloc_pad, padw], FP16)
    h_full = nc.dram_tensor("h_full", [ncore * nloc_pad, padw], FP16,
                            addr_space="Shared")
    p1_full = nc.dram_tensor("p1_full", [ncore * nloc_pad, padw], FP16,
                             addr_space="Shared")

    with tile.TileContext(nc, num_cores=ncore) as tc:
        with tc.tile_pool(name="res", bufs=1) as res:
            # ---------------- resident loads ----------------
            def load(pool, dram, shape, dt, name):
                t = pool.tile(shape, dt, name=name, tag=name)
                nc.sync.dma_start(out=t[:], in_=dram[:])
                return t

            w1a = load(res, w1_d[0:P, :], [P, hid], FP16, "w1a")
            w1b = load(res, w1_d[P:in_f + 1, :], [in_f + 1 - P, hid], FP16, "w1b")
            w2a = load(res, w2_d[0:P, :], [P, hid], FP16, "w2a")
            w2b = load(res, w2_d[P:hid + 1, :], [hid + 1 - P, hid], FP16, "w2b")
            gi_s = load(res, gi_d[:, :], [P, nperm], I16, "gi_s")
            do_s = load(res, do_d[:, :], [P, G_total], F32, "do_s")
            ew_s = load(res, ew_d[:, :], [P, G_total], F32, "ew_s")
            tidx_s = load(res, tidx_d[:, :], [P, MC // 16], I16, "tidx_s")
            wca = [load(res, wcomb_d[k * hid:k * hid + P, :], [P, 4 * hid], FP16,
                        f"wca{k}") for k in range(3)]
            wcb = [load(res, wcomb_d[k * hid + P:(k + 1) * hid, :], [h2, 4 * hid],
                        FP16, f"wcb{k}") for k in range(3)]
            wcbias = load(res, wcomb_d[3 * hid:3 * hid + 1, :], [1, 4 * hid], FP16,
                          "wcbias")
            wxa = load(res, wxp_d[0:P, :], [P, hid], FP16, "wxa")
            wxb = load(res, wxp_d[P:hid, :], [h2, hid], FP16, "wxb")
            wxbias = load(res, wxp_d[hid:hid + 1, :], [1, hid], FP16, "wxbias")
            wya = [load(res, wy_d[k * hid:k * hid + P, :], [P, 3 * out_f], FP16,
                        f"wya{k}") for k in range(3)]
            wyb = [load(res, wy_d[k * hid + P:(k + 1) * hid, :], [h2, 3 * out_f],
                        FP16, f"wyb{k}") for k in range(3)]
            cb_s = load(res, cb_d, [P, 12], F32, "cb_s")
            bfcb_s = load(res, bfcb_d, [P, 4 * out_f], F32, "bfcb_s")

            ones1 = res.tile([1, P], FP16)
            nc.vector.memset(ones1[:], 1.0)
            outbuf = res.tile([P, cfg.tiles * out_f], F32)

            # iota row 0..127 on every partition (exact in fp16)
            iota_s = res.tile([P, P], FP16, name="iota_s")
            nc.gpsimd.iota(iota_s[:], pattern=[[1, P]], base=0,
                           channel_multiplier=0,
                           allow_small_or_imprecise_dtypes=True)

            # gather buffer ring — memset once so slots skipped by negative
            # indices stay finite (sel multiplies them by 0)
            g_bufs = [res.tile([P, cfg.gcap, padw], FP16, name=f"gbuf{i}")
                      for i in range(cfg.gbufs)]
            for gb in g_bufs:
                nc.vector.memset(gb[:], 0.0)

            # ---------------- MLP: h = relu(x@W1+b1)@W2+b2 ----------------
            with (
                tc.tile_pool(name="mlp", bufs=3) as mlp,
                tc.tile_pool(name="psm", bufs=2, space="PSUM") as psA,
                tc.tile_pool(name="psm2", bufs=2, space="PSUM") as psB,
            ):
              for m0 in range(0, nloc_pad, MC):
                  F = min(MC, nloc_pad - m0)
                  xa = mlp.tile([P, F], FP16, tag="xa")
                  xb = mlp.tile([in_f + 1 - P, F], FP16, tag="xb")
                  nc.sync.dma_start(out=xa[:], in_=xT_d[0:P, m0:m0 + F])
                  nc.sync.dma_start(out=xb[:], in_=xT_d[P:in_f + 1, m0:m0 + F])
                  pa = psA.tile([P, F], F32, tag="mlp_pa")
                  pb = psA.tile([h2, F], F32, tag="mlp_pb")
                  nc.tensor.matmul(pa[:], w1a[:, 0:P], xa[:], start=True, stop=False)
                  nc.tensor.matmul(pa[:], w1b[:, 0:P], xb[:], start=False, stop=True)
                  nc.tensor.matmul(pb[:], w1a[:, P:hid], xa[:], start=True, stop=False)
                  nc.tensor.matmul(pb[:], w1b[:, P:hid], xb[:], start=False, stop=True)
                  t1a = mlp.tile([P, F], FP16, tag="t1a")
                  t1b = mlp.tile([hid + 1 - P, F], FP16, tag="t1b")
                  nc.scalar.activation(t1a[:], pa[:], ACT.Relu)
                  nc.vector.memset(t1b[:], 1.0)
                  nc.scalar.activation(t1b[0:h2, :], pb[:], ACT.Relu)
                  for s0 in range(0, F, P):
                      hp = psB.tile([P, hid], F32, tag="mlp_hp")
                      nc.tensor.matmul(hp[:], t1a[:, s0:s0 + P], w2a[:],
                                       start=True, stop=False)
                      nc.tensor.matmul(hp[:], t1b[:, s0:s0 + P], w2b[:],
                                       start=False, stop=True)
                      hrow = mlp.tile([P, hid], FP16, tag="hrow")
                      nc.scalar.activation(hrow[:], hp[:], ACT.Copy)
                      nc.sync.dma_start(
                          out=h_tab[m0 + s0:m0 + s0 + P, 0:hid], in_=hrow[:])

            # ---------------- propagation (segment-sum via matmul) --------
            # plan entries grouped per block
            blk_entries = {}
            eoff = 0
            for (b, q, g, nvalid) in plan:
                blk_entries.setdefault(b, []).append((q, g, nvalid, eoff))
                eoff += g

            gctr = [0]

            def prop(gat, psp, src_full, dst_tab):
                qsl = [src_full[q * QROWS:min((q + 1) * QROWS,
                                              ncore * nloc_pad), :]
                       for q in range(nq)]
                qn = 0
                for b in range(cfg.tiles):
                    entries = blk_entries.get(b, [])
                    ps = psp.tile([P, hid], F32, tag="segp")
                    nmm = sum(g for (_, g, _, _) in entries)
                    mi = 0
                    for (q, g, nvalid, goff) in entries:
                        gb = g_bufs[gctr[0] % cfg.gbufs]
                        gctr[0] += 1
                        i16 = goff * 8  # = goff*128/16
                        nc.gpsimd.dma_gather(
                            gb[:, 0:g, :], qsl[q],
                            gi_s[:, i16:i16 + g * 8],
                            g * P, nvalid, padw, queue_num=qn % 4)
                        qn += 1
                        for k in range(g):
                            gcol = goff + k
                            sel = gat.tile([P, P], FP16, tag="sel")
                            nc.vector.tensor_scalar(
                                sel[:], iota_s[:],
                                do_s[:, gcol:gcol + 1],
                                ew_s[:, gcol:gcol + 1],
                                ALU.is_equal, ALU.mult)
                            nc.tensor.matmul(ps[:], sel[:], gb[:, k, 0:hid],
                                             start=(mi == 0),
                                             stop=(mi == nmm - 1))
                            mi += 1
                    ob = gat.tile([P, hid], FP16, tag="ob")
                    if nmm == 0:
                        nc.vector.memset(ob[:], 0.0)
                    else:
                        nc.scalar.activation(ob[:], ps[:], ACT.Copy)
                    nc.sync.dma_start(
                        out=dst_tab[b * P:(b + 1) * P, 0:hid], in_=ob[:])

            with (
                tc.tile_pool(name="gat", bufs=6) as gat,
                tc.tile_pool(name="psp", bufs=3, space="PSUM") as psp,
            ):
                nc.gpsimd.collective_compute(
                    "AllGather", ALU.bypass, replica_groups=rg,
                    ins=[h_tab[:]], outs=[h_full[:]])
                prop(gat, psp, h_full, p1_tab)
                nc.gpsimd.collective_compute(
                    "AllGather", ALU.bypass, replica_groups=rg,
                    ins=[p1_tab[:]], outs=[p1_full[:]])
                prop(gat, psp, p1_full, p2_tab)

            # ---------------- epilogue ----------------
            with (
                tc.tile_pool(name="epi", bufs=2) as epi,
                tc.tile_pool(name="pse", bufs=2, space="PSUM") as psA,
                tc.tile_pool(name="pse2", bufs=2, space="PSUM") as psB,
            ):
              for m0 in range(0, nloc_pad, MC):
                  F = min(MC, nloc_pad - m0)
                  tT = []
                  for ti, tab in enumerate((h_tab, p1_tab, p2_tab)):
                      t = epi.tile([P, padw // P, F], FP16, tag=f"tT{ti}",
                                   name=f"tT{ti}")
                      nc.gpsimd.dma_gather(
                          t[:], tab[m0:m0 + F, :],
                          tidx_s[:, 0:F // 16], F, F, padw, transpose=True,
                          queue_num=ti % 4)
                      tT.append(t)
                  for s0 in range(0, F, P):
                      sl = slice(s0, s0 + P)
                      pc = [psA.tile([P, 2 * hid], F32, tag=f"comb{i}",
                                     name=f"comb{i}") for i in range(2)]
                      for i in range(2):
                          cs = slice(i * 2 * hid, (i + 1) * 2 * hid)
                          for k in range(3):
                              nc.tensor.matmul(pc[i][:], tT[k][:, 0, sl],
                                               wca[k][:, cs],
                                               start=(k == 0), stop=False)
                              nc.tensor.matmul(pc[i][:], tT[k][0:h2, 1, sl],
                                               wcb[k][:, cs],
                                               start=False, stop=False)
                          nc.tensor.matmul(pc[i][:], ones1[:], wcbias[:, cs],
                                           start=False, stop=True)
                      px = psB.tile([P, hid], F32, tag="px")
                      nc.tensor.matmul(px[:], tT[0][:, 0, sl], wxa[:],
                                       start=True, stop=False)
                      nc.tensor.matmul(px[:], tT[0][0:h2, 1, sl], wxb[:],
                                       start=False, stop=False)
                      nc.tensor.matmul(px[:], ones1[:], wxbias[:],
                                       start=False, stop=True)
                      py = psB.tile([P, 3 * out_f], F32, tag="py")
                      for k in range(3):
                          nc.tensor.matmul(py[:], tT[k][:, 0, sl], wya[k][:],
                                           start=(k == 0), stop=False)
                          nc.tensor.matmul(py[:], tT[k][0:h2, 1, sl], wyb[k][:],
                                           start=False, stop=(k == 2))
                      hp0 = epi.tile([P, 2 * hid], FP16, tag="hp0")
                      hp1 = epi.tile([P, 2 * hid], FP16, tag="hp1")
                      xp = epi.tile([P, hid], FP16, tag="xp")
                      nc.scalar.activation(hp0[:], pc[0][:], ACT.Tanh)
                      nc.scalar.activation(hp1[:], pc[1][:], ACT.Tanh)
                      nc.scalar.activation(xp[:], px[:], ACT.Tanh)
                      scr = epi.tile([P, hid], F32, tag="scr")
                      logit = epi.tile([P, 4], F32, tag="logit")
                      for f in range(4):
                          hsrc = (hp0, hp1)[f // 2]
                          nc.vector.tensor_mul(
                              scr[:],
                              hsrc[:, (f % 2) * hid:(f % 2 + 1) * hid], xp[:])
                          nc.vector.tensor_reduce(
                              logit[:, f:f + 1], scr[:], AX.X, ALU.add)
                      mxn = epi.tile([P, 1], F32, tag="mxn")
                      nc.vector.tensor_reduce(mxn[:], logit[:], AX.X, ALU.max,
                                              negate=True)
                      el = epi.tile([P, 4], F32, tag="el")
                      nc.scalar.activation(el[:], logit[:], ACT.Exp, bias=mxn[:, 0:1])
                      sm = epi.tile([P, 1], F32, tag="sm")
                      nc.vector.tensor_reduce(sm[:], el[:], AX.X, ALU.add)
                      rs = epi.tile([P, 1], F32, tag="rs")
                      nc.vector.reciprocal(rs[:], sm[:])
                      score = epi.tile([P, 4], F32, tag="score")
                      nc.vector.tensor_scalar_mul(score[:], el[:], rs[:, 0:1])
                      scr4 = epi.tile([P, 4], F32, tag="scr4")
                      wk = epi.tile([P, 3], F32, tag="wk")
                      sbf = epi.tile([P, out_f], F32, tag="sbf")
                      for k in range(3):
                          nc.vector.tensor_mul(scr4[:], score[:],
                                               cb_s[:, k * 4:(k + 1) * 4])
                          nc.vector.tensor_reduce(wk[:, k:k + 1], scr4[:],
                                                  AX.X, ALU.add)
                      for j in range(out_f):
                          nc.vector.tensor_mul(scr4[:], score[:],
                                               bfcb_s[:, j * 4:(j + 1) * 4])
                          nc.vector.tensor_reduce(sbf[:, j:j + 1], scr4[:],
                                                  AX.X, ALU.add)
                      tgl = (m0 + s0) // P
                      ob = outbuf[:, tgl * out_f:(tgl + 1) * out_f]
                      scr2 = epi.tile([P, out_f], F32, tag="scr2")
                      nc.vector.tensor_scalar_mul(ob, py[:, 0:out_f], wk[:, 0:1])
                      nc.vector.tensor_scalar_mul(scr2[:], py[:, out_f:2 * out_f],
                                                  wk[:, 1:2])
                      nc.vector.tensor_add(ob, ob, scr2[:])
                      nc.vector.tensor_scalar_mul(scr2[:], py[:, 2 * out_f:3 * out_f],
                                                  wk[:, 2:3])
                      nc.vector.tensor_add(ob, ob, scr2[:])
                      nc.vector.tensor_add(ob, ob, sbf[:])

            nc.sync.dma_start(
                out=out_d[:].rearrange("(t p) j -> p t j", p=P),
                in_=outbuf[:].rearrange("p (t j) -> p t j", j=out_f))
    nc.compile()
    return nc


def numpy_model(cfg, in_maps, G_total, plan):
    """Bit-approximate numpy model of what the device computes (f32 math),
    for validating the kernel structure without hardware."""
    ncore, nloc_pad, hid, out_f = cfg.ncore, cfg.nloc_pad, cfg.hid, cfg.out_f
    B = G_total * P
    outs = []
    h_tabs = []
    for c in range(ncore):
        im = in_maps[c]
        xT = im["xT"].astype(np.float32)
        w1 = im["w1"].astype(np.float32)
        w2 = im["w2"].astype(np.float32)
        t1 = np.maximum(xT.T @ w1, 0.0)
        t1 = np.concatenate([t1, np.ones((nloc_pad, 1), np.float32)], 1)
        h = t1 @ w2
        h_tabs.append(h.astype(np.float16).astype(np.float32))

    def prop_all(tabs):
        full = np.concatenate(tabs, 0)
        res = []
        for c in range(ncore):
            im = in_maps[c]
            out = np.zeros((nloc_pad, hid), np.float32)
            gi_f = im["gi"][:16, :].T.reshape(-1)
            do_f = im["do"].T.reshape(-1)
            ew_f = im["ew"].T.reshape(-1)
            off = 0
            for (b, q, g, _nv) in plan:
                ch = g * P
                gidx = gi_f[off:off + ch].astype(np.int64)
                dof = do_f[off:off + ch]
                ewf = ew_f[off:off + ch]
                m = gidx >= 0
                rows = np.zeros((ch, hid), np.float32)
                rows[m] = full[q * QROWS + gidx[m], :].astype(
                    np.float16).astype(np.float32)
                dloc = dof.astype(np.int64)
                valid = (dloc >= 0) & (dloc < P)
                acc = np.zeros((P, hid), np.float32)
                np.add.at(acc, dloc[valid],
                          (ewf[valid, None].astype(np.float16).astype(np.float32)
                           * rows[valid]))
                out[b * P:(b + 1) * P] += acc
                off += ch
            res.append(out.astype(np.float16).astype(np.float32))
        return res

    p1_tabs = prop_all(h_tabs)
    p2_tabs = prop_all(p1_tabs)

    for c in range(ncore):
        im = in_maps[c]
        hT = h_tabs[c]
        p1 = p1_tabs[c]
        p2 = p2_tabs[c]
        stack = np.concatenate([hT, p1, p2, np.ones((nloc_pad, 1), np.float32)], 1)
        comb = stack @ im["wcomb"].astype(np.float32)
        hproj = np.tanh(comb).reshape(nloc_pad, 4, hid)
        xp = np.tanh(np.concatenate([hT, np.ones((nloc_pad, 1), np.float32)], 1)
                     @ im["wxp"].astype(np.float32))
        logits = np.einsum("nfd,nd->nf", hproj, xp)
        e = np.exp(logits - logits.max(1, keepdims=True))
        score = e / e.sum(1, keepdims=True)
        y = (stack[:, :3 * hid] @ im["wy"].astype(np.float32)).reshape(
            nloc_pad, 3, out_f)
        wk = np.stack([(score * im["cb"][0, k * 4:(k + 1) * 4][None, :]).sum(1)
                       for k in range(3)], 1)
        sbf = np.stack([(score * im["bfcb"][0, j * 4:(j + 1) * 4][None, :]).sum(1)
                        for j in range(out_f)], 1)
        out = (y * wk[:, :, None]).sum(1) + sbf
        outs.append(out.astype(np.float32))
    return outs


# ---------------------------------------------------------------------------
# Self-contained harness entry point: kernel(**inputs) -> np.ndarray
# ---------------------------------------------------------------------------
_NC_CACHE = {}


def kernel(**inputs):
    """AMNet forward on 8 TRN2 NeuronCores. Takes full unsharded inputs,
    returns the full [N, 2] float32 output."""
    from concourse.bass_utils import run_bass_kernel_spmd

    cfg = Cfg(n=100000, ncore=8, in_f=166, hid=156, out_f=2)
    in_maps, G_total, plan = host_preprocess(cfg, **inputs)
    key = (G_total, tuple(plan))
    nc = _NC_CACHE.get(key)
    if nc is None:
        nc = build(cfg, G_total, plan)
        _NC_CACHE[key] = nc
    res = run_bass_kernel_spmd(nc, in_maps,
                               core_ids=list(range(cfg.ncore)), trace=False)
    out = np.concatenate(
        [res.results[i]["out"][:cfg.nloc] for i in range(cfg.ncore)], 0)
    return out.astype(np.float32)


# revision 12
# speedup vs baseline: 2.1615x; 1.2574x over previous
"""AMNet (BernNet-style GNN) distributed Bass kernel for 8 TRN2 NeuronCores.

Math reformulation (K=2 Bernstein basis):
  reference does 5 sparse props; but with p0 = h, p1 = A_hat h, p2 = A_hat p1:
    B0 = (p0 + 2 p1 + p2)/4,  B1 = (p0 - p2)/2,  B2 = (p0 - 2 p1 + p2)/4
  so only TWO sparse propagations are needed.
  filters: filt_f = sum_k c[f,k] p_k + b_filt[f],  c = relu(theta) @ M
  attention epilogue fully refactored into matmuls (see build()).

Distribution: nodes sharded over 8 cores (12500 each). Edges partitioned by
dst core. Each prop: AllGather the (padded fp16) node table to h_full, then
per 128-dst block: dma_gather the source rows (sorted by dst block), build a
weighted one-hot selector on DVE (sel[e,d] = ew[e] * (dstoff[e]==d)) and
segment-reduce on TensorE into a PSUM tile — NO dma_scatter_add, no
accumulator tables, no merge phase. Block results stream out sequentially.
"""

import math

import numpy as np

import concourse.bass as bass
import concourse.tile as tile
from concourse import bacc, library_config, mybir

FP16 = mybir.dt.float16
F32 = mybir.dt.float32
I16 = mybir.dt.int16
P = 128
AX = mybir.AxisListType
ALU = mybir.AluOpType
ACT = mybir.ActivationFunctionType

QROWS = 32768          # gather source slice rows (int16 index range)


def _patch_swdge_lane_assignment():
    """Tile round-robins DMASW sem lanes ignoring queue_num, but each lane is
    locked to one SWDGE queue by the ucode/sim. Pin lane = queue_num + 4*flip
    so multi-queue swdge DMAs get consistent lanes (8 lanes / 4 queues)."""
    import concourse.tile_sem_assignment as tsa
    if getattr(tsa, "_amnet_lane_patch", False):
        return
    tsa._amnet_lane_patch = True
    orig = tsa.TileClockTick._assign_tick

    def _assign_tick(self, inst):
        if (isinstance(inst, tsa.DMAInst)
                and inst.engine == tsa.mybir.EngineType.Pool
                and not isinstance(inst, tsa.bass_isa.UserSyncedRemoteDMADescs)):
            q = getattr(inst, "queue_num", 0) or 0
            flips = getattr(self, "_amnet_qflip", None)
            if flips is None:
                flips = self._amnet_qflip = [0, 0, 0, 0]
            lane = q + 4 * flips[q]
            flips[q] ^= 1
            save = self.next_sw_dma_idx
            self.next_sw_dma_idx = lane
            try:
                return orig(self, inst)
            finally:
                self.next_sw_dma_idx = save
        return orig(self, inst)

    tsa.TileClockTick._assign_tick = _assign_tick


_patch_swdge_lane_assignment()


class Cfg:
    def __init__(self, n, ncore, in_f, hid, out_f):
        assert n % ncore == 0
        self.n = n
        self.ncore = ncore
        self.nloc = n // ncore
        self.in_f = in_f
        self.hid = hid
        self.out_f = out_f
        self.nloc_pad = ((self.nloc + P - 1) // P) * P
        self.tiles = self.nloc_pad // P       # dst blocks per core
        self.padw = 256            # fp16 table row elems (512B, 256B-multiple)
        self.gcap = 8              # dma_gather num_idxs cap is 1024 (HW)
        self.gbufs = 8             # gather buffer ring size
        # MLP node-chunk size (PSUM free limit 512)
        self.mlp_chunk = 512


def _wrap16(a, pad_val, total):
    """idx array -> [128, total//16] int16 in the dma_gather wrapped layout."""
    out = np.full(total, pad_val, dtype=np.int16)
    out[: a.shape[0]] = a.astype(np.int16)
    w = out.reshape(total // 16, 16).T  # elem j -> [j%16, j//16]
    return np.tile(w, (8, 1)).copy()   # replicated for the 8 gpsimd cores


def host_preprocess(cfg, x, edge_index, W1, b1, W2, b2, theta, b_filt,
                    Wf, bf, Wx, bx, Wc, bc):
    """Build per-core input maps. Returns (in_maps, G_total, plan)."""
    n, ncore, nloc = cfg.n, cfg.ncore, cfg.nloc
    nloc_pad, hid, in_f, out_f = cfg.nloc_pad, cfg.hid, cfg.in_f, cfg.out_f

    src = np.asarray(edge_index[0], dtype=np.int64)
    dst = np.asarray(edge_index[1], dtype=np.int64)
    deg = np.bincount(dst, minlength=n).astype(np.float32)
    dinv = (1.0 / np.sqrt(np.maximum(deg, 1.0))).astype(np.float32)
    ewv = dinv[src] * dinv[dst]

    # global row in the concatenated (padded) full table
    src_row = (src // nloc) * nloc_pad + (src % nloc)
    nq = (ncore * nloc_pad + QROWS - 1) // QROWS

    ecore = dst // nloc

    # Per core: sort edges by (dst block, src slice q, src_row).
    per_core = []
    cnts = np.zeros((ncore, cfg.tiles, nq), np.int64)
    for c in range(ncore):
        sel = ecore == c
        es_row, ed, ev = src_row[sel], dst[sel] - c * nloc, ewv[sel]
        blk = ed // P
        q = es_row // QROWS
        o = np.lexsort((es_row, q, blk))
        es_row, ed, ev, blk, q = es_row[o], ed[o], ev[o], blk[o], q[o]
        per_core.append((es_row, ed, ev))
        np.add.at(cnts[c], (blk, q), 1)

    # plan: [(block, q, ngroups, nvalid)] identical across cores; per-entry
    # slot counts are maxed over cores and rounded up to whole 128-groups.
    # nvalid = number of non-skipped descriptors in the call (the gather
    # ucode requires num_idxs_reg == count of non-negative indices, so every
    # core pads its real edges with idx=0 descriptors up to nvalid).
    maxc = cnts.max(axis=0)  # [tiles, nq]
    plan = []
    for b in range(cfg.tiles):
        for q in range(nq):
            mc = int(maxc[b, q])
            g = (mc + P - 1) // P
            off = 0
            while g > 0:
                take = min(g, cfg.gcap)
                nvalid = min(mc - off, take * P)
                plan.append((b, q, take, nvalid))
                off += take * P
                g -= take
    G_total = sum(g for (_, _, g, _) in plan)
    B = G_total * P

    # ---- weights ----
    h2 = hid - P
    thr = np.maximum(np.asarray(theta, np.float64), 0.0)           # relu
    M = np.array([[.25, .5, .25], [.5, 0., -.5], [.25, -.5, .25]], np.float64)
    c3 = (thr @ M)                                                 # [4,3]

    W1 = np.asarray(W1, np.float64); W2 = np.asarray(W2, np.float64)
    Wf = np.asarray(Wf, np.float64); Wx = np.asarray(Wx, np.float64)
    Wc = np.asarray(Wc, np.float64)
    b1 = np.asarray(b1, np.float64); b2 = np.asarray(b2, np.float64)
    bf = np.asarray(bf, np.float64); bx = np.asarray(bx, np.float64)
    bc = np.asarray(bc, np.float64); bflt = np.asarray(b_filt, np.float64)

    w1p = np.concatenate([W1, b1[None, :]], 0).astype(np.float16)      # [in_f+1, hid]
    w2p = np.concatenate([W2, b2[None, :]], 0).astype(np.float16)      # [hid+1, hid]

    wcomb = np.zeros((3 * hid + 1, 4 * hid), np.float64)
    for f in range(4):
        for k in range(3):
            wcomb[k * hid:(k + 1) * hid, f * hid:(f + 1) * hid] = c3[f, k] * Wf
        wcomb[3 * hid, f * hid:(f + 1) * hid] = bflt[f] @ Wf + bf
    wcomb = wcomb.astype(np.float16)

    wxp = np.concatenate([Wx, bx[None, :]], 0).astype(np.float16)      # [hid+1, hid]

    wy = np.zeros((3 * hid, 3 * out_f), np.float64)
    for k in range(3):
        wy[k * hid:(k + 1) * hid, k * out_f:(k + 1) * out_f] = Wc
    wy = wy.astype(np.float16)

    cbm = np.zeros((P, 12), np.float32)          # cols k*4+f = c3[f,k]
    for k in range(3):
        for f in range(4):
            cbm[:, k * 4 + f] = c3[f, k]
    bfc = bflt @ Wc                              # [4, out_f]
    bfcb = np.zeros((P, 4 * out_f), np.float32)  # cols j*4+f = bfc[f,j]+bc[j]
    for j in range(out_f):
        for f in range(4):
            bfcb[:, j * 4 + f] = bfc[f, j] + bc[j]

    tidx = _wrap16(np.arange(cfg.mlp_chunk), 0, cfg.mlp_chunk)

    x = np.asarray(x, np.float32)

    in_maps = []
    for c in range(ncore):
        xT = np.zeros((in_f + 1, nloc_pad), np.float16)
        xT[:in_f, :nloc] = x[c * nloc:(c + 1) * nloc].T
        xT[in_f, :] = 1.0

        es_row, ed, ev = per_core[c]
        blk = ed // P
        q = es_row // QROWS
        # per (b, q) run boundaries in the sorted arrays
        # fill flat slot arrays per plan entry
        gflat = np.full(B, -1, np.int64)       # -1 = skipped descriptor
        doflat = np.full(B, -1.0, np.float32)  # -1 = sel matches nothing
        ewflat = np.zeros(B, np.float32)
        key = blk * nq + q
        starts = np.searchsorted(key, np.arange(cfg.tiles * nq), side="left")
        ends = np.searchsorted(key, np.arange(cfg.tiles * nq), side="right")
        consumed = {}
        off = 0
        for (b, qq, g, nvalid) in plan:
            kidx = b * nq + qq
            s0, s1 = int(starts[kidx]), int(ends[kidx])
            done = consumed.get(kidx, 0)
            take = max(0, min(g * P, (s1 - s0) - done))
            if take > 0:
                sl = slice(s0 + done, s0 + done + take)
                gflat[off:off + take] = es_row[sl] - qq * QROWS
                doflat[off:off + take] = (ed[sl] - b * P).astype(np.float32)
                ewflat[off:off + take] = ev[sl]
            # pad with idx=0 descriptors (sel-zeroed) up to the call's
            # shared valid count; the rest stay -1 (skipped)
            if nvalid > take:
                gflat[off + take:off + nvalid] = 0
            consumed[kidx] = done + take
            off += g * P
        assert off == B

        gi = np.tile(gflat.astype(np.int16).reshape(B // 16, 16).T,
                     (8, 1)).copy()
        # slot j of group g -> partition j%128; DVE tables are
        # [P, G_total] with column g holding slots [g*128, (g+1)*128)
        dot = doflat.reshape(G_total, P).T.astype(np.float16).copy()
        ewt = ewflat.reshape(G_total, P).T.astype(np.float16).copy()

        in_maps.append({
            "xT": xT,
            "w1": w1p, "w2": w2p, "wcomb": wcomb, "wxp": wxp, "wy": wy,
            "cb": cbm, "bfcb": bfcb,
            "gi": gi, "do": dot, "ew": ewt, "tidx": tidx,
        })
    return in_maps, G_total, plan


def build(cfg, G_total, plan):
    """Build the SPMD Bass graph. All cores run this same program."""
    ncore, nloc_pad, hid, in_f, out_f, padw = (
        cfg.ncore, cfg.nloc_pad, cfg.hid, cfg.in_f, cfg.out_f, cfg.padw)
    B = G_total * P
    nperm = B // 16
    rg = [list(range(ncore))]
    h2 = hid - P            # 28
    MC = cfg.mlp_chunk
    nq = (ncore * nloc_pad + QROWS - 1) // QROWS

    nc = bacc.Bacc(None, num_devices=ncore, num_swdge_queues=4)

    dp = nc.declare_dram_parameter
    xT_d = dp("xT", [in_f + 1, nloc_pad], FP16, isOutput=False)
    w1_d = dp("w1", [in_f + 1, hid], FP16, isOutput=False)
    w2_d = dp("w2", [hid + 1, hid], FP16, isOutput=False)
    wcomb_d = dp("wcomb", [3 * hid + 1, 4 * hid], FP16, isOutput=False)
    wxp_d = dp("wxp", [hid + 1, hid], FP16, isOutput=False)
    wy_d = dp("wy", [3 * hid, 3 * out_f], FP16, isOutput=False)
    cb_d = dp("cb", [P, 12], F32, isOutput=False)
    bfcb_d = dp("bfcb", [P, 4 * out_f], F32, isOutput=False)
    gi_d = dp("gi", [P, nperm], I16, isOutput=False)
    do_d = dp("do", [P, G_total], FP16, isOutput=False)
    ew_d = dp("ew", [P, G_total], FP16, isOutput=False)
    tidx_d = dp("tidx", [P, MC // 16], I16, isOutput=False)
    out_d = dp("out", [nloc_pad, out_f], F32, isOutput=True)

    h_tab = nc.dram_tensor("h_tab", [nloc_pad, padw], FP16)
    p1_tab = nc.dram_tensor("p1_tab", [nloc_pad, padw], FP16)
    p2_tab = nc.dram_tensor("p2_tab", [nloc_pad, padw], FP16)
    h_full = nc.dram_tensor("h_full", [ncore * nloc_pad, padw], FP16,
                            addr_space="Shared")
    p1_full = nc.dram_tensor("p1_full", [ncore * nloc_pad, padw], FP16,
                             addr_space="Shared")

    with tile.TileContext(nc, num_cores=ncore) as tc:
        with tc.tile_pool(name="res", bufs=1) as res:
            # ---------------- resident loads ----------------
            def load(pool, dram, shape, dt, name):
                t = pool.tile(shape, dt, name=name, tag=name)
                nc.sync.dma_start(out=t[:], in_=dram[:])
                return t

            w1a = load(res, w1_d[0:P, :], [P, hid], FP16, "w1a")
            w1b = load(res, w1_d[P:in_f + 1, :], [in_f + 1 - P, hid], FP16, "w1b")
            w2a = load(res, w2_d[0:P, :], [P, hid], FP16, "w2a")
            w2b = load(res, w2_d[P:hid + 1, :], [hid + 1 - P, hid], FP16, "w2b")
            gi_s = load(res, gi_d[:, :], [P, nperm], I16, "gi_s")
            do_s = load(res, do_d[:, :], [P, G_total], FP16, "do_s")
            ew_s = load(res, ew_d[:, :], [P, G_total], FP16, "ew_s")
            tidx_s = load(res, tidx_d[:, :], [P, MC // 16], I16, "tidx_s")
            wca = [load(res, wcomb_d[k * hid:k * hid + P, :], [P, 4 * hid], FP16,
                        f"wca{k}") for k in range(3)]
            wcb = [load(res, wcomb_d[k * hid + P:(k + 1) * hid, :], [h2, 4 * hid],
                        FP16, f"wcb{k}") for k in range(3)]
            wcbias = load(res, wcomb_d[3 * hid:3 * hid + 1, :], [1, 4 * hid], FP16,
                          "wcbias")
            wxa = load(res, wxp_d[0:P, :], [P, hid], FP16, "wxa")
            wxb = load(res, wxp_d[P:hid, :], [h2, hid], FP16, "wxb")
            wxbias = load(res, wxp_d[hid:hid + 1, :], [1, hid], FP16, "wxbias")
            wya = [load(res, wy_d[k * hid:k * hid + P, :], [P, 3 * out_f], FP16,
                        f"wya{k}") for k in range(3)]
            wyb = [load(res, wy_d[k * hid + P:(k + 1) * hid, :], [h2, 3 * out_f],
                        FP16, f"wyb{k}") for k in range(3)]
            cb_s = load(res, cb_d, [P, 12], F32, "cb_s")
            bfcb_s = load(res, bfcb_d, [P, 4 * out_f], F32, "bfcb_s")

            ones1 = res.tile([1, P], FP16)
            nc.vector.memset(ones1[:], 1.0)
            outbuf = res.tile([P, cfg.tiles * out_f], F32)

            # iota row 0..127 on every partition (exact in fp16)
            iota_s = res.tile([P, P], FP16, name="iota_s")
            nc.gpsimd.iota(iota_s[:], pattern=[[1, P]], base=0,
                           channel_multiplier=0,
                           allow_small_or_imprecise_dtypes=True)

            # gather buffer ring — memset once so slots skipped by negative
            # indices stay finite (sel multiplies them by 0)
            g_bufs = [res.tile([P, cfg.gcap, padw], FP16, name=f"gbuf{i}")
                      for i in range(cfg.gbufs)]
            for gb in g_bufs:
                nc.vector.memset(gb[:], 0.0)

            # ---------------- MLP: h = relu(x@W1+b1)@W2+b2 ----------------
            with (
                tc.tile_pool(name="mlp", bufs=3) as mlp,
                tc.tile_pool(name="psm", bufs=2, space="PSUM") as psA,
                tc.tile_pool(name="psm2", bufs=2, space="PSUM") as psB,
            ):
              for m0 in range(0, nloc_pad, MC):
                  F = min(MC, nloc_pad - m0)
                  xa = mlp.tile([P, F], FP16, tag="xa")
                  xb = mlp.tile([in_f + 1 - P, F], FP16, tag="xb")
                  nc.sync.dma_start(out=xa[:], in_=xT_d[0:P, m0:m0 + F])
                  nc.sync.dma_start(out=xb[:], in_=xT_d[P:in_f + 1, m0:m0 + F])
                  pa = psA.tile([P, F], F32, tag="mlp_pa")
                  pb = psA.tile([h2, F], F32, tag="mlp_pb")
                  nc.tensor.matmul(pa[:], w1a[:, 0:P], xa[:], start=True, stop=False)
                  nc.tensor.matmul(pa[:], w1b[:, 0:P], xb[:], start=False, stop=True)
                  nc.tensor.matmul(pb[:], w1a[:, P:hid], xa[:], start=True, stop=False)
                  nc.tensor.matmul(pb[:], w1b[:, P:hid], xb[:], start=False, stop=True)
                  t1a = mlp.tile([P, F], FP16, tag="t1a")
                  t1b = mlp.tile([hid + 1 - P, F], FP16, tag="t1b")
                  nc.scalar.activation(t1a[:], pa[:], ACT.Relu)
                  nc.vector.memset(t1b[:], 1.0)
                  nc.scalar.activation(t1b[0:h2, :], pb[:], ACT.Relu)
                  for s0 in range(0, F, P):
                      hp = psB.tile([P, hid], F32, tag="mlp_hp")
                      nc.tensor.matmul(hp[:], t1a[:, s0:s0 + P], w2a[:],
                                       start=True, stop=False)
                      nc.tensor.matmul(hp[:], t1b[:, s0:s0 + P], w2b[:],
                                       start=False, stop=True)
                      hrow = mlp.tile([P, hid], FP16, tag="hrow")
                      nc.scalar.activation(hrow[:], hp[:], ACT.Copy)
                      nc.sync.dma_start(
                          out=h_tab[m0 + s0:m0 + s0 + P, 0:hid], in_=hrow[:])

            # ---------------- propagation (segment-sum via matmul) --------
            # plan entries grouped per block
            blk_entries = {}
            eoff = 0
            for (b, q, g, nvalid) in plan:
                blk_entries.setdefault(b, []).append((q, g, nvalid, eoff))
                eoff += g

            gctr = [0]
            maxg = max(sum(g for (_, g, _, _) in es)
                       for es in blk_entries.values())

            def prop(gat, psp, src_full, dst_tab):
                qsl = [src_full[q * QROWS:min((q + 1) * QROWS,
                                              ncore * nloc_pad), :]
                       for q in range(nq)]
                qn = 0
                for b in range(cfg.tiles):
                    entries = blk_entries.get(b, [])
                    ps = psp.tile([P, hid], F32, tag="segp")
                    nmm = sum(g for (_, g, _, _) in entries)
                    g0 = entries[0][3] if entries else 0
                    # batched selector build: sel[p, k, d] =
                    #   ew[p, g0+k] * (iota[d] == dstoff[p, g0+k])
                    sel = gat.tile([P, nmm, P], FP16, tag="sel",
                                   padded_shape=[P, maxg, P])
                    nc.vector.tensor_tensor(
                        out=sel[:],
                        in0=iota_s[:].unsqueeze(1).broadcast_to((P, nmm, P)),
                        in1=do_s[:, g0:g0 + nmm].to_broadcast((P, nmm, P)),
                        op=ALU.is_equal)
                    nc.vector.tensor_tensor(
                        out=sel[:], in0=sel[:],
                        in1=ew_s[:, g0:g0 + nmm].to_broadcast((P, nmm, P)),
                        op=ALU.mult)
                    mi = 0
                    for (q, g, nvalid, goff) in entries:
                        gb = g_bufs[gctr[0] % cfg.gbufs]
                        gctr[0] += 1
                        i16 = goff * 8  # = goff*128/16
                        nc.gpsimd.dma_gather(
                            gb[:, 0:g, :], qsl[q],
                            gi_s[:, i16:i16 + g * 8],
                            g * P, nvalid, padw, queue_num=qn % 4)
                        qn += 1
                        for k in range(g):
                            nc.tensor.matmul(ps[:], sel[:, goff - g0 + k, :],
                                             gb[:, k, 0:hid],
                                             start=(mi == 0),
                                             stop=(mi == nmm - 1))
                            mi += 1
                    ob = gat.tile([P, hid], FP16, tag="ob")
                    if nmm == 0:
                        nc.vector.memset(ob[:], 0.0)
                    else:
                        nc.scalar.activation(ob[:], ps[:], ACT.Copy)
                    nc.sync.dma_start(
                        out=dst_tab[b * P:(b + 1) * P, 0:hid], in_=ob[:])

            with (
                tc.tile_pool(name="gat", bufs=6) as gat,
                tc.tile_pool(name="psp", bufs=3, space="PSUM") as psp,
            ):
                nc.gpsimd.collective_compute(
                    "AllGather", ALU.bypass, replica_groups=rg,
                    ins=[h_tab[:]], outs=[h_full[:]])
                prop(gat, psp, h_full, p1_tab)
                nc.gpsimd.collective_compute(
                    "AllGather", ALU.bypass, replica_groups=rg,
                    ins=[p1_tab[:]], outs=[p1_full[:]])
                prop(gat, psp, p1_full, p2_tab)

            # ---------------- epilogue ----------------
            with (
                tc.tile_pool(name="epi", bufs=2) as epi,
                tc.tile_pool(name="pse", bufs=2, space="PSUM") as psA,
                tc.tile_pool(name="pse2", bufs=2, space="PSUM") as psB,
            ):
              for m0 in range(0, nloc_pad, MC):
                  F = min(MC, nloc_pad - m0)
                  tT = []
                  for ti, tab in enumerate((h_tab, p1_tab, p2_tab)):
                      t = epi.tile([P, padw // P, F], FP16, tag=f"tT{ti}",
                                   name=f"tT{ti}")
                      nc.gpsimd.dma_gather(
                          t[:], tab[m0:m0 + F, :],
                          tidx_s[:, 0:F // 16], F, F, padw, transpose=True,
                          queue_num=ti % 4)
                      tT.append(t)
                  for s0 in range(0, F, P):
                      sl = slice(s0, s0 + P)
                      pc = [psA.tile([P, 2 * hid], F32, tag=f"comb{i}",
                                     name=f"comb{i}") for i in range(2)]
                      for i in range(2):
                          cs = slice(i * 2 * hid, (i + 1) * 2 * hid)
                          for k in range(3):
                              nc.tensor.matmul(pc[i][:], tT[k][:, 0, sl],
                                               wca[k][:, cs],
                                               start=(k == 0), stop=False)
                              nc.tensor.matmul(pc[i][:], tT[k][0:h2, 1, sl],
                                               wcb[k][:, cs],
                                               start=False, stop=False)
                          nc.tensor.matmul(pc[i][:], ones1[:], wcbias[:, cs],
                                           start=False, stop=True)
                      px = psB.tile([P, hid], F32, tag="px")
                      nc.tensor.matmul(px[:], tT[0][:, 0, sl], wxa[:],
                                       start=True, stop=False)
                      nc.tensor.matmul(px[:], tT[0][0:h2, 1, sl], wxb[:],
                                       start=False, stop=False)
                      nc.tensor.matmul(px[:], ones1[:], wxbias[:],
                                       start=False, stop=True)
                      py = psB.tile([P, 3 * out_f], F32, tag="py")
                      for k in range(3):
                          nc.tensor.matmul(py[:], tT[k][:, 0, sl], wya[k][:],
                                           start=(k == 0), stop=False)
                          nc.tensor.matmul(py[:], tT[k][0:h2, 1, sl], wyb[k][:],
                                           start=False, stop=(k == 2))
                      hp0 = epi.tile([P, 2 * hid], FP16, tag="hp0")
                      hp1 = epi.tile([P, 2 * hid], FP16, tag="hp1")
                      xp = epi.tile([P, hid], FP16, tag="xp")
                      nc.scalar.activation(hp0[:], pc[0][:], ACT.Tanh)
                      nc.scalar.activation(hp1[:], pc[1][:], ACT.Tanh)
                      nc.scalar.activation(xp[:], px[:], ACT.Tanh)
                      scr = epi.tile([P, hid], F32, tag="scr")
                      logit = epi.tile([P, 4], F32, tag="logit")
                      for f in range(4):
                          hsrc = (hp0, hp1)[f // 2]
                          nc.vector.tensor_mul(
                              scr[:],
                              hsrc[:, (f % 2) * hid:(f % 2 + 1) * hid], xp[:])
                          nc.vector.tensor_reduce(
                              logit[:, f:f + 1], scr[:], AX.X, ALU.add)
                      mxn = epi.tile([P, 1], F32, tag="mxn")
                      nc.vector.tensor_reduce(mxn[:], logit[:], AX.X, ALU.max,
                                              negate=True)
                      el = epi.tile([P, 4], F32, tag="el")
                      nc.scalar.activation(el[:], logit[:], ACT.Exp, bias=mxn[:, 0:1])
                      sm = epi.tile([P, 1], F32, tag="sm")
                      nc.vector.tensor_reduce(sm[:], el[:], AX.X, ALU.add)
                      rs = epi.tile([P, 1], F32, tag="rs")
                      nc.vector.reciprocal(rs[:], sm[:])
                      score = epi.tile([P, 4], F32, tag="score")
                      nc.vector.tensor_scalar_mul(score[:], el[:], rs[:, 0:1])
                      scr4 = epi.tile([P, 4], F32, tag="scr4")
                      wk = epi.tile([P, 3], F32, tag="wk")
                      sbf = epi.tile([P, out_f], F32, tag="sbf")
                      for k in range(3):
                          nc.vector.tensor_mul(scr4[:], score[:],
                                               cb_s[:, k * 4:(k + 1) * 4])
                          nc.vector.tensor_reduce(wk[:, k:k + 1], scr4[:],
                                                  AX.X, ALU.add)
                      for j in range(out_f):
                          nc.vector.tensor_mul(scr4[:], score[:],
                                               bfcb_s[:, j * 4:(j + 1) * 4])
                          nc.vector.tensor_reduce(sbf[:, j:j + 1], scr4[:],
                                                  AX.X, ALU.add)
                      tgl = (m0 + s0) // P
                      ob = outbuf[:, tgl * out_f:(tgl + 1) * out_f]
                      scr2 = epi.tile([P, out_f], F32, tag="scr2")
                      nc.vector.tensor_scalar_mul(ob, py[:, 0:out_f], wk[:, 0:1])
                      nc.vector.tensor_scalar_mul(scr2[:], py[:, out_f:2 * out_f],
                                                  wk[:, 1:2])
                      nc.vector.tensor_add(ob, ob, scr2[:])
                      nc.vector.tensor_scalar_mul(scr2[:], py[:, 2 * out_f:3 * out_f],
                                                  wk[:, 2:3])
                      nc.vector.tensor_add(ob, ob, scr2[:])
                      nc.vector.tensor_add(ob, ob, sbf[:])

            nc.sync.dma_start(
                out=out_d[:].rearrange("(t p) j -> p t j", p=P),
                in_=outbuf[:].rearrange("p (t j) -> p t j", j=out_f))
    nc.compile()
    return nc


def numpy_model(cfg, in_maps, G_total, plan):
    """Bit-approximate numpy model of what the device computes (f32 math),
    for validating the kernel structure without hardware."""
    ncore, nloc_pad, hid, out_f = cfg.ncore, cfg.nloc_pad, cfg.hid, cfg.out_f
    B = G_total * P
    outs = []
    h_tabs = []
    for c in range(ncore):
        im = in_maps[c]
        xT = im["xT"].astype(np.float32)
        w1 = im["w1"].astype(np.float32)
        w2 = im["w2"].astype(np.float32)
        t1 = np.maximum(xT.T @ w1, 0.0)
        t1 = np.concatenate([t1, np.ones((nloc_pad, 1), np.float32)], 1)
        h = t1 @ w2
        h_tabs.append(h.astype(np.float16).astype(np.float32))

    def prop_all(tabs):
        full = np.concatenate(tabs, 0)
        res = []
        for c in range(ncore):
            im = in_maps[c]
            out = np.zeros((nloc_pad, hid), np.float32)
            gi_f = im["gi"][:16, :].T.reshape(-1)
            do_f = im["do"].T.reshape(-1)
            ew_f = im["ew"].T.reshape(-1)
            off = 0
            for (b, q, g, _nv) in plan:
                ch = g * P
                gidx = gi_f[off:off + ch].astype(np.int64)
                dof = do_f[off:off + ch]
                ewf = ew_f[off:off + ch]
                m = gidx >= 0
                rows = np.zeros((ch, hid), np.float32)
                rows[m] = full[q * QROWS + gidx[m], :].astype(
                    np.float16).astype(np.float32)
                dloc = dof.astype(np.int64)
                valid = (dloc >= 0) & (dloc < P)
                acc = np.zeros((P, hid), np.float32)
                np.add.at(acc, dloc[valid],
                          (ewf[valid, None].astype(np.float16).astype(np.float32)
                           * rows[valid]))
                out[b * P:(b + 1) * P] += acc
                off += ch
            res.append(out.astype(np.float16).astype(np.float32))
        return res

    p1_tabs = prop_all(h_tabs)
    p2_tabs = prop_all(p1_tabs)

    for c in range(ncore):
        im = in_maps[c]
        hT = h_tabs[c]
        p1 = p1_tabs[c]
        p2 = p2_tabs[c]
        stack = np.concatenate([hT, p1, p2, np.ones((nloc_pad, 1), np.float32)], 1)
        comb = stack @ im["wcomb"].astype(np.float32)
        hproj = np.tanh(comb).reshape(nloc_pad, 4, hid)
        xp = np.tanh(np.concatenate([hT, np.ones((nloc_pad, 1), np.float32)], 1)
                     @ im["wxp"].astype(np.float32))
        logits = np.einsum("nfd,nd->nf", hproj, xp)
        e = np.exp(logits - logits.max(1, keepdims=True))
        score = e / e.sum(1, keepdims=True)
        y = (stack[:, :3 * hid] @ im["wy"].astype(np.float32)).reshape(
            nloc_pad, 3, out_f)
        wk = np.stack([(score * im["cb"][0, k * 4:(k + 1) * 4][None, :]).sum(1)
                       for k in range(3)], 1)
        sbf = np.stack([(score * im["bfcb"][0, j * 4:(j + 1) * 4][None, :]).sum(1)
                        for j in range(out_f)], 1)
        out = (y * wk[:, :, None]).sum(1) + sbf
        outs.append(out.astype(np.float32))
    return outs


# ---------------------------------------------------------------------------
# Self-contained harness entry point: kernel(**inputs) -> np.ndarray
# ---------------------------------------------------------------------------
_NC_CACHE = {}


def kernel(**inputs):
    """AMNet forward on 8 TRN2 NeuronCores. Takes full unsharded inputs,
    returns the full [N, 2] float32 output."""
    from concourse.bass_utils import run_bass_kernel_spmd

    cfg = Cfg(n=100000, ncore=8, in_f=166, hid=156, out_f=2)
    in_maps, G_total, plan = host_preprocess(cfg, **inputs)
    key = (G_total, tuple(plan))
    nc = _NC_CACHE.get(key)
    if nc is None:
        nc = build(cfg, G_total, plan)
        _NC_CACHE[key] = nc
    res = run_bass_kernel_spmd(nc, in_maps,
                               core_ids=list(range(cfg.ncore)), trace=False)
    out = np.concatenate(
        [res.results[i]["out"][:cfg.nloc] for i in range(cfg.ncore)], 0)
    return out.astype(np.float32)


# revision 19
# speedup vs baseline: 2.2612x; 1.0461x over previous
"""AMNet (BernNet-style GNN) distributed Bass kernel for 8 TRN2 NeuronCores.

Math reformulation (K=2 Bernstein basis):
  reference does 5 sparse props; but with p0 = h, p1 = A_hat h, p2 = A_hat p1:
    B0 = (p0 + 2 p1 + p2)/4,  B1 = (p0 - p2)/2,  B2 = (p0 - 2 p1 + p2)/4
  so only TWO sparse propagations are needed.
  filters: filt_f = sum_k c[f,k] p_k + b_filt[f],  c = relu(theta) @ M
  attention epilogue fully refactored into matmuls (see build()).

Distribution: nodes sharded over 8 cores (12500 each). Edges partitioned by
dst core. Each prop: AllGather the (padded fp16) node table to h_full, then
per 128-dst block: dma_gather the source rows (sorted by dst block), build a
weighted one-hot selector on DVE (sel[e,d] = ew[e] * (dstoff[e]==d)) and
segment-reduce on TensorE into a PSUM tile — NO dma_scatter_add, no
accumulator tables, no merge phase. Block results stream out sequentially.
"""

import math

import numpy as np

import concourse.bass as bass
import concourse.tile as tile
from concourse import bacc, library_config, mybir

FP16 = mybir.dt.float16
F32 = mybir.dt.float32
I16 = mybir.dt.int16
P = 128
AX = mybir.AxisListType
ALU = mybir.AluOpType
ACT = mybir.ActivationFunctionType

QROWS = 32768          # gather source slice rows (int16 index range)
NSLC = 7               # AllGather slices (nloc_pad must divide evenly)


def _patch_swdge_lane_assignment():
    """Tile round-robins DMASW sem lanes ignoring queue_num, but each lane is
    locked to one SWDGE queue by the ucode/sim. Pin lane = queue_num + 4*flip
    so multi-queue swdge DMAs get consistent lanes (8 lanes / 4 queues)."""
    import concourse.tile_sem_assignment as tsa
    if getattr(tsa, "_amnet_lane_patch", False):
        return
    tsa._amnet_lane_patch = True
    orig = tsa.TileClockTick._assign_tick

    def _assign_tick(self, inst):
        if (isinstance(inst, tsa.DMAInst)
                and inst.engine == tsa.mybir.EngineType.Pool
                and not isinstance(inst, tsa.bass_isa.UserSyncedRemoteDMADescs)):
            q = getattr(inst, "queue_num", 0) or 0
            flips = getattr(self, "_amnet_qflip", None)
            if flips is None:
                flips = self._amnet_qflip = [0, 0, 0, 0]
            lane = q + 4 * flips[q]
            flips[q] ^= 1
            save = self.next_sw_dma_idx
            self.next_sw_dma_idx = lane
            try:
                return orig(self, inst)
            finally:
                self.next_sw_dma_idx = save
        return orig(self, inst)

    tsa.TileClockTick._assign_tick = _assign_tick


_patch_swdge_lane_assignment()


class Cfg:
    def __init__(self, n, ncore, in_f, hid, out_f):
        assert n % ncore == 0
        self.n = n
        self.ncore = ncore
        self.nloc = n // ncore
        self.in_f = in_f
        self.hid = hid
        self.out_f = out_f
        self.nloc_pad = ((self.nloc + P - 1) // P) * P
        self.tiles = self.nloc_pad // P       # dst blocks per core
        self.padw = 256            # fp16 table row elems (512B, 256B-multiple)
        self.gcap = 8              # dma_gather num_idxs cap is 1024 (HW)
        self.gbufs = 8             # gather buffer ring size
        # MLP node-chunk size (PSUM free limit 512)
        self.mlp_chunk = 512


def _wrap16(a, pad_val, total):
    """idx array -> [128, total//16] int16 in the dma_gather wrapped layout."""
    out = np.full(total, pad_val, dtype=np.int16)
    out[: a.shape[0]] = a.astype(np.int16)
    w = out.reshape(total // 16, 16).T  # elem j -> [j%16, j//16]
    return np.tile(w, (8, 1)).copy()   # replicated for the 8 gpsimd cores


def host_preprocess(cfg, x, edge_index, W1, b1, W2, b2, theta, b_filt,
                    Wf, bf, Wx, bx, Wc, bc):
    """Build per-core input maps. Returns (in_maps, G_total, plan)."""
    n, ncore, nloc = cfg.n, cfg.ncore, cfg.nloc
    nloc_pad, hid, in_f, out_f = cfg.nloc_pad, cfg.hid, cfg.in_f, cfg.out_f

    src = np.asarray(edge_index[0], dtype=np.int64)
    dst = np.asarray(edge_index[1], dtype=np.int64)
    deg = np.bincount(dst, minlength=n).astype(np.float32)
    dinv = (1.0 / np.sqrt(np.maximum(deg, 1.0))).astype(np.float32)
    ewv = dinv[src] * dinv[dst]

    # global row in the slice-major full table: the AllGather is emitted in
    # NSLC row-slices, so the full table is laid out [slice][core][row]
    R = nloc_pad // NSLC
    sc, sr = src // nloc, src % nloc
    src_row = (sr // R) * (ncore * R) + sc * R + (sr % R)
    nq = (ncore * nloc_pad + QROWS - 1) // QROWS

    ecore = dst // nloc

    # Per core: sort edges by (dst block, src slice q, src_row).
    per_core = []
    cnts = np.zeros((ncore, cfg.tiles, nq), np.int64)
    for c in range(ncore):
        sel = ecore == c
        es_row, ed, ev = src_row[sel], dst[sel] - c * nloc, ewv[sel]
        blk = ed // P
        q = es_row // QROWS
        o = np.lexsort((es_row, q, blk))
        es_row, ed, ev, blk, q = es_row[o], ed[o], ev[o], blk[o], q[o]
        per_core.append((es_row, ed, ev))
        np.add.at(cnts[c], (blk, q), 1)

    # plan: [(block, q, ngroups, nvalid)] identical across cores; per-entry
    # slot counts are maxed over cores and rounded up to whole 128-groups.
    # nvalid = number of non-skipped descriptors in the call (the gather
    # ucode requires num_idxs_reg == count of non-negative indices, so every
    # core pads its real edges with idx=0 descriptors up to nvalid).
    maxc = cnts.max(axis=0)  # [tiles, nq]
    plan = []
    for b in range(cfg.tiles):
        for q in range(nq):
            mc = int(maxc[b, q])
            g = (mc + P - 1) // P
            off = 0
            while g > 0:
                take = min(g, cfg.gcap)
                nvalid = min(mc - off, take * P)
                plan.append((b, q, take, nvalid))
                off += take * P
                g -= take
    G_total = sum(g for (_, _, g, _) in plan)
    B = G_total * P

    # ---- weights ----
    h2 = hid - P
    thr = np.maximum(np.asarray(theta, np.float64), 0.0)           # relu
    M = np.array([[.25, .5, .25], [.5, 0., -.5], [.25, -.5, .25]], np.float64)
    c3 = (thr @ M)                                                 # [4,3]

    W1 = np.asarray(W1, np.float64); W2 = np.asarray(W2, np.float64)
    Wf = np.asarray(Wf, np.float64); Wx = np.asarray(Wx, np.float64)
    Wc = np.asarray(Wc, np.float64)
    b1 = np.asarray(b1, np.float64); b2 = np.asarray(b2, np.float64)
    bf = np.asarray(bf, np.float64); bx = np.asarray(bx, np.float64)
    bc = np.asarray(bc, np.float64); bflt = np.asarray(b_filt, np.float64)

    w1p = np.concatenate([W1, b1[None, :]], 0).astype(np.float16)      # [in_f+1, hid]
    w2p = np.concatenate([W2, b2[None, :]], 0).astype(np.float16)      # [hid+1, hid]

    wcomb = np.zeros((3 * hid + 1, 4 * hid), np.float64)
    for f in range(4):
        for k in range(3):
            wcomb[k * hid:(k + 1) * hid, f * hid:(f + 1) * hid] = c3[f, k] * Wf
        wcomb[3 * hid, f * hid:(f + 1) * hid] = bflt[f] @ Wf + bf
    wcomb = wcomb.astype(np.float16)

    wxp = np.concatenate([Wx, bx[None, :]], 0).astype(np.float16)      # [hid+1, hid]

    wy = np.zeros((3 * hid, 3 * out_f), np.float64)
    for k in range(3):
        wy[k * hid:(k + 1) * hid, k * out_f:(k + 1) * out_f] = Wc
    wy = wy.astype(np.float16)

    cbm = np.zeros((P, 12), np.float32)          # cols k*4+f = c3[f,k]
    for k in range(3):
        for f in range(4):
            cbm[:, k * 4 + f] = c3[f, k]
    bfc = bflt @ Wc                              # [4, out_f]
    bfcb = np.zeros((P, 4 * out_f), np.float32)  # cols j*4+f = bfc[f,j]+bc[j]
    for j in range(out_f):
        for f in range(4):
            bfcb[:, j * 4 + f] = bfc[f, j] + bc[j]

    tidx = _wrap16(np.arange(cfg.mlp_chunk), 0, cfg.mlp_chunk)

    x = np.asarray(x, np.float32)

    in_maps = []
    for c in range(ncore):
        xT = np.zeros((in_f + 1, nloc_pad), np.float16)
        xT[:in_f, :nloc] = x[c * nloc:(c + 1) * nloc].T
        xT[in_f, :] = 1.0

        es_row, ed, ev = per_core[c]
        blk = ed // P
        q = es_row // QROWS
        # per (b, q) run boundaries in the sorted arrays
        # fill flat slot arrays per plan entry
        gflat = np.full(B, -1, np.int64)       # -1 = skipped descriptor
        doflat = np.full(B, -1.0, np.float32)  # -1 = sel matches nothing
        ewflat = np.zeros(B, np.float32)
        key = blk * nq + q
        starts = np.searchsorted(key, np.arange(cfg.tiles * nq), side="left")
        ends = np.searchsorted(key, np.arange(cfg.tiles * nq), side="right")
        consumed = {}
        off = 0
        for (b, qq, g, nvalid) in plan:
            kidx = b * nq + qq
            s0, s1 = int(starts[kidx]), int(ends[kidx])
            done = consumed.get(kidx, 0)
            take = max(0, min(g * P, (s1 - s0) - done))
            if take > 0:
                sl = slice(s0 + done, s0 + done + take)
                gflat[off:off + take] = es_row[sl] - qq * QROWS
                doflat[off:off + take] = (ed[sl] - b * P).astype(np.float32)
                ewflat[off:off + take] = ev[sl]
            # pad with idx=0 descriptors (sel-zeroed) up to the call's
            # shared valid count; the rest stay -1 (skipped)
            if nvalid > take:
                gflat[off + take:off + nvalid] = 0
            consumed[kidx] = done + take
            off += g * P
        assert off == B

        gi = np.tile(gflat.astype(np.int16).reshape(B // 16, 16).T,
                     (8, 1)).copy()
        # slot j of group g -> partition j%128; DVE tables are
        # [P, G_total] with column g holding slots [g*128, (g+1)*128)
        dot = doflat.reshape(G_total, P).T.astype(np.float16).copy()
        ewt = ewflat.reshape(G_total, P).T.astype(np.float16).copy()

        in_maps.append({
            "xT": xT,
            "w1": w1p, "w2": w2p, "wcomb": wcomb, "wxp": wxp, "wy": wy,
            "cb": cbm, "bfcb": bfcb,
            "gi": gi, "do": dot, "ew": ewt, "tidx": tidx,
        })
    return in_maps, G_total, plan


def build(cfg, G_total, plan):
    """Build the SPMD Bass graph. All cores run this same program."""
    ncore, nloc_pad, hid, in_f, out_f, padw = (
        cfg.ncore, cfg.nloc_pad, cfg.hid, cfg.in_f, cfg.out_f, cfg.padw)
    B = G_total * P
    nperm = B // 16
    rg = [list(range(ncore))]
    h2 = hid - P            # 28
    MC = cfg.mlp_chunk
    nq = (ncore * nloc_pad + QROWS - 1) // QROWS

    nc = bacc.Bacc(None, num_devices=ncore, num_swdge_queues=4)

    dp = nc.declare_dram_parameter
    xT_d = dp("xT", [in_f + 1, nloc_pad], FP16, isOutput=False)
    w1_d = dp("w1", [in_f + 1, hid], FP16, isOutput=False)
    w2_d = dp("w2", [hid + 1, hid], FP16, isOutput=False)
    wcomb_d = dp("wcomb", [3 * hid + 1, 4 * hid], FP16, isOutput=False)
    wxp_d = dp("wxp", [hid + 1, hid], FP16, isOutput=False)
    wy_d = dp("wy", [3 * hid, 3 * out_f], FP16, isOutput=False)
    cb_d = dp("cb", [P, 12], F32, isOutput=False)
    bfcb_d = dp("bfcb", [P, 4 * out_f], F32, isOutput=False)
    gi_d = dp("gi", [P, nperm], I16, isOutput=False)
    do_d = dp("do", [P, G_total], FP16, isOutput=False)
    ew_d = dp("ew", [P, G_total], FP16, isOutput=False)
    tidx_d = dp("tidx", [P, MC // 16], I16, isOutput=False)
    out_d = dp("out", [nloc_pad, out_f], F32, isOutput=True)

    h_tab = nc.dram_tensor("h_tab", [nloc_pad, padw], FP16)
    p1_tab = nc.dram_tensor("p1_tab", [nloc_pad, padw], FP16)
    p2_tab = nc.dram_tensor("p2_tab", [nloc_pad, padw], FP16)
    h_full = nc.dram_tensor("h_full", [ncore * nloc_pad, padw], FP16,
                            addr_space="Shared")
    p1_full = nc.dram_tensor("p1_full", [ncore * nloc_pad, padw], FP16,
                             addr_space="Shared")

    with tile.TileContext(nc, num_cores=ncore) as tc:
        with tc.tile_pool(name="res", bufs=1) as res:
            # ---------------- resident loads ----------------
            def load(pool, dram, shape, dt, name):
                t = pool.tile(shape, dt, name=name, tag=name)
                nc.sync.dma_start(out=t[:], in_=dram[:])
                return t

            w1a = load(res, w1_d[0:P, :], [P, hid], FP16, "w1a")
            w1b = load(res, w1_d[P:in_f + 1, :], [in_f + 1 - P, hid], FP16, "w1b")
            w2a = load(res, w2_d[0:P, :], [P, hid], FP16, "w2a")
            w2b = load(res, w2_d[P:hid + 1, :], [hid + 1 - P, hid], FP16, "w2b")
            gi_s = load(res, gi_d[:, :], [P, nperm], I16, "gi_s")
            do_s = load(res, do_d[:, :], [P, G_total], FP16, "do_s")
            ew_s = load(res, ew_d[:, :], [P, G_total], FP16, "ew_s")
            tidx_s = load(res, tidx_d[:, :], [P, MC // 16], I16, "tidx_s")
            wca = [load(res, wcomb_d[k * hid:k * hid + P, :], [P, 4 * hid], FP16,
                        f"wca{k}") for k in range(3)]
            wcb = [load(res, wcomb_d[k * hid + P:(k + 1) * hid, :], [h2, 4 * hid],
                        FP16, f"wcb{k}") for k in range(3)]
            wcbias = load(res, wcomb_d[3 * hid:3 * hid + 1, :], [1, 4 * hid], FP16,
                          "wcbias")
            wxa = load(res, wxp_d[0:P, :], [P, hid], FP16, "wxa")
            wxb = load(res, wxp_d[P:hid, :], [h2, hid], FP16, "wxb")
            wxbias = load(res, wxp_d[hid:hid + 1, :], [1, hid], FP16, "wxbias")
            wya = [load(res, wy_d[k * hid:k * hid + P, :], [P, 3 * out_f], FP16,
                        f"wya{k}") for k in range(3)]
            wyb = [load(res, wy_d[k * hid + P:(k + 1) * hid, :], [h2, 3 * out_f],
                        FP16, f"wyb{k}") for k in range(3)]
            cb_s = load(res, cb_d, [P, 12], F32, "cb_s")
            bfcb_s = load(res, bfcb_d, [P, 4 * out_f], F32, "bfcb_s")

            ones1 = res.tile([1, P], FP16)
            nc.vector.memset(ones1[:], 1.0)
            outbuf = res.tile([P, cfg.tiles * out_f], F32)

            # iota row 0..127 on every partition (exact in fp16)
            iota_s = res.tile([P, P], FP16, name="iota_s")
            nc.gpsimd.iota(iota_s[:], pattern=[[1, P]], base=0,
                           channel_multiplier=0,
                           allow_small_or_imprecise_dtypes=True)

            # gather buffer ring — memset once so slots skipped by negative
            # indices stay finite (sel multiplies them by 0)
            g_bufs = [res.tile([P, cfg.gcap, padw], FP16, name=f"gbuf{i}")
                      for i in range(cfg.gbufs)]
            for gb in g_bufs:
                nc.vector.memset(gb[:], 0.0)

            # sliced AllGather: slice s covers local rows [s*R, (s+1)*R) and
            # lands contiguously at full[(s*ncore + core)*R, ...]
            R = nloc_pad // NSLC

            def ag_slice(src_tab, dst_full, s):
                nc.gpsimd.collective_compute(
                    "AllGather", ALU.bypass, replica_groups=rg,
                    ins=[src_tab[s * R:(s + 1) * R, :]],
                    outs=[dst_full[s * ncore * R:(s + 1) * ncore * R, :]])

            # ---------------- MLP: h = relu(x@W1+b1)@W2+b2 ----------------
            with (
                tc.tile_pool(name="mlp", bufs=3) as mlp,
                tc.tile_pool(name="psm", bufs=2, space="PSUM") as psA,
                tc.tile_pool(name="psm2", bufs=2, space="PSUM") as psB,
            ):
              ag_done = 0
              for m0 in range(0, nloc_pad, MC):
                  F = min(MC, nloc_pad - m0)
                  xa = mlp.tile([P, F], FP16, tag="xa")
                  xb = mlp.tile([in_f + 1 - P, F], FP16, tag="xb")
                  nc.sync.dma_start(out=xa[:], in_=xT_d[0:P, m0:m0 + F])
                  nc.sync.dma_start(out=xb[:], in_=xT_d[P:in_f + 1, m0:m0 + F])
                  pa = psA.tile([P, F], F32, tag="mlp_pa")
                  pb = psA.tile([h2, F], F32, tag="mlp_pb")
                  nc.tensor.matmul(pa[:], w1a[:, 0:P], xa[:], start=True, stop=False)
                  nc.tensor.matmul(pa[:], w1b[:, 0:P], xb[:], start=False, stop=True)
                  nc.tensor.matmul(pb[:], w1a[:, P:hid], xa[:], start=True, stop=False)
                  nc.tensor.matmul(pb[:], w1b[:, P:hid], xb[:], start=False, stop=True)
                  t1a = mlp.tile([P, F], FP16, tag="t1a")
                  t1b = mlp.tile([hid + 1 - P, F], FP16, tag="t1b")
                  nc.scalar.activation(t1a[:], pa[:], ACT.Relu)
                  nc.vector.memset(t1b[:], 1.0)
                  nc.scalar.activation(t1b[0:h2, :], pb[:], ACT.Relu)
                  for s0 in range(0, F, P):
                      hp = psB.tile([P, hid], F32, tag="mlp_hp")
                      nc.tensor.matmul(hp[:], t1a[:, s0:s0 + P], w2a[:],
                                       start=True, stop=False)
                      nc.tensor.matmul(hp[:], t1b[:, s0:s0 + P], w2b[:],
                                       start=False, stop=True)
                      hrow = mlp.tile([P, hid], FP16, tag="hrow")
                      nc.scalar.activation(hrow[:], hp[:], ACT.Copy)
                      nc.sync.dma_start(
                          out=h_tab[m0 + s0:m0 + s0 + P, 0:hid], in_=hrow[:])
                  while (ag_done + 1) * R <= m0 + F:
                      ag_slice(h_tab, h_full, ag_done)
                      ag_done += 1
              assert ag_done == NSLC

            # ---------------- propagation (segment-sum via matmul) --------
            # plan entries grouped per block
            blk_entries = {}
            eoff = 0
            for (b, q, g, nvalid) in plan:
                blk_entries.setdefault(b, []).append((q, g, nvalid, eoff))
                eoff += g

            gctr = [0]
            maxg = max(sum(g for (_, g, _, _) in es)
                       for es in blk_entries.values())

            def prop(gat, psp, src_full, dst_tab, ag_dst=None):
                qsl = [src_full[q * QROWS:min((q + 1) * QROWS,
                                              ncore * nloc_pad), :]
                       for q in range(nq)]
                qn = 0
                blk_per_slice = (nloc_pad // P) // NSLC
                for b in range(cfg.tiles):
                    entries = blk_entries.get(b, [])
                    nmm = sum(g for (_, g, _, _) in entries)
                    # two PSUM accumulators per block (even/odd groups) so
                    # consecutive matmuls alternate banks
                    nps = 2 if nmm > 1 else 1
                    ps = [psp.tile([P, hid], F32, tag=f"segp{i}",
                                   name=f"segp{i}")
                          for i in range(nps)]
                    started = [False] * nps
                    g0 = entries[0][3] if entries else 0
                    # batched selector build: sel[p, k, d] =
                    #   ew[p, g0+k] * (iota[d] == dstoff[p, g0+k])
                    sel = gat.tile([P, nmm, P], FP16, tag="sel",
                                   padded_shape=[P, maxg, P])
                    nc.vector.tensor_tensor(
                        out=sel[:],
                        in0=iota_s[:].unsqueeze(1).broadcast_to((P, nmm, P)),
                        in1=do_s[:, g0:g0 + nmm].to_broadcast((P, nmm, P)),
                        op=ALU.is_equal)
                    nc.vector.tensor_tensor(
                        out=sel[:], in0=sel[:],
                        in1=ew_s[:, g0:g0 + nmm].to_broadcast((P, nmm, P)),
                        op=ALU.mult)
                    mi = 0
                    for (q, g, nvalid, goff) in entries:
                        gb = g_bufs[gctr[0] % cfg.gbufs]
                        gctr[0] += 1
                        i16 = goff * 8  # = goff*128/16
                        nc.gpsimd.dma_gather(
                            gb[:, 0:g, :], qsl[q],
                            gi_s[:, i16:i16 + g * 8],
                            g * P, nvalid, padw, queue_num=qn % 4)
                        qn += 1
                        for k in range(g):
                            pi = mi % nps
                            nc.tensor.matmul(ps[pi][:], sel[:, goff - g0 + k, :],
                                             gb[:, k, 0:hid],
                                             start=(not started[pi]),
                                             stop=(mi >= nmm - nps))
                            started[pi] = True
                            mi += 1
                    ob = gat.tile([P, hid], FP16, tag="ob")
                    if nmm == 0:
                        nc.vector.memset(ob[:], 0.0)
                    elif nps == 1:
                        nc.scalar.activation(ob[:], ps[0][:], ACT.Copy)
                    else:
                        nc.scalar.activation(ob[:], ps[0][:], ACT.Copy)
                        nc.vector.tensor_add(ob[:], ob[:], ps[1][:])
                    nc.sync.dma_start(
                        out=dst_tab[b * P:(b + 1) * P, 0:hid], in_=ob[:])
                    if ag_dst is not None and (b + 1) % blk_per_slice == 0:
                        ag_slice(dst_tab, ag_dst, (b + 1) // blk_per_slice - 1)

            with (
                tc.tile_pool(name="gat", bufs=6) as gat,
                tc.tile_pool(name="psp", bufs=4, space="PSUM") as psp,
            ):
                prop(gat, psp, h_full, p1_tab, ag_dst=p1_full)
                prop(gat, psp, p1_full, p2_tab)

            # ---------------- epilogue ----------------
            with (
                tc.tile_pool(name="epi", bufs=2) as epi,
                tc.tile_pool(name="pse", bufs=2, space="PSUM") as psA,
                tc.tile_pool(name="pse2", bufs=2, space="PSUM") as psB,
            ):
              for m0 in range(0, nloc_pad, MC):
                  F = min(MC, nloc_pad - m0)
                  tT = []
                  for ti, tab in enumerate((h_tab, p1_tab, p2_tab)):
                      t = epi.tile([P, padw // P, F], FP16, tag=f"tT{ti}",
                                   name=f"tT{ti}")
                      nc.gpsimd.dma_gather(
                          t[:], tab[m0:m0 + F, :],
                          tidx_s[:, 0:F // 16], F, F, padw, transpose=True,
                          queue_num=ti % 4)
                      tT.append(t)
                  for s0 in range(0, F, P):
                      sl = slice(s0, s0 + P)
                      pc = [psA.tile([P, 2 * hid], F32, tag=f"comb{i}",
                                     name=f"comb{i}") for i in range(2)]
                      for i in range(2):
                          cs = slice(i * 2 * hid, (i + 1) * 2 * hid)
                          for k in range(3):
                              nc.tensor.matmul(pc[i][:], tT[k][:, 0, sl],
                                               wca[k][:, cs],
                                               start=(k == 0), stop=False)
                              nc.tensor.matmul(pc[i][:], tT[k][0:h2, 1, sl],
                                               wcb[k][:, cs],
                                               start=False, stop=False)
                          nc.tensor.matmul(pc[i][:], ones1[:], wcbias[:, cs],
                                           start=False, stop=True)
                      px = psB.tile([P, hid], F32, tag="px")
                      nc.tensor.matmul(px[:], tT[0][:, 0, sl], wxa[:],
                                       start=True, stop=False)
                      nc.tensor.matmul(px[:], tT[0][0:h2, 1, sl], wxb[:],
                                       start=False, stop=False)
                      nc.tensor.matmul(px[:], ones1[:], wxbias[:],
                                       start=False, stop=True)
                      py = psB.tile([P, 3 * out_f], F32, tag="py")
                      for k in range(3):
                          nc.tensor.matmul(py[:], tT[k][:, 0, sl], wya[k][:],
                                           start=(k == 0), stop=False)
                          nc.tensor.matmul(py[:], tT[k][0:h2, 1, sl], wyb[k][:],
                                           start=False, stop=(k == 2))
                      hp0 = epi.tile([P, 2 * hid], FP16, tag="hp0")
                      hp1 = epi.tile([P, 2 * hid], FP16, tag="hp1")
                      xp = epi.tile([P, hid], FP16, tag="xp")
                      nc.scalar.activation(hp0[:], pc[0][:], ACT.Tanh)
                      nc.scalar.activation(hp1[:], pc[1][:], ACT.Tanh)
                      nc.scalar.activation(xp[:], px[:], ACT.Tanh)
                      scr = epi.tile([P, hid], F32, tag="scr")
                      logit = epi.tile([P, 4], F32, tag="logit")
                      for f in range(4):
                          hsrc = (hp0, hp1)[f // 2]
                          nc.vector.tensor_mul(
                              scr[:],
                              hsrc[:, (f % 2) * hid:(f % 2 + 1) * hid], xp[:])
                          nc.vector.tensor_reduce(
                              logit[:, f:f + 1], scr[:], AX.X, ALU.add)
                      mxn = epi.tile([P, 1], F32, tag="mxn")
                      nc.vector.tensor_reduce(mxn[:], logit[:], AX.X, ALU.max,
                                              negate=True)
                      el = epi.tile([P, 4], F32, tag="el")
                      nc.scalar.activation(el[:], logit[:], ACT.Exp, bias=mxn[:, 0:1])
                      sm = epi.tile([P, 1], F32, tag="sm")
                      nc.vector.tensor_reduce(sm[:], el[:], AX.X, ALU.add)
                      rs = epi.tile([P, 1], F32, tag="rs")
                      nc.vector.reciprocal(rs[:], sm[:])
                      score = epi.tile([P, 4], F32, tag="score")
                      nc.vector.tensor_scalar_mul(score[:], el[:], rs[:, 0:1])
                      scr4 = epi.tile([P, 4], F32, tag="scr4")
                      wk = epi.tile([P, 3], F32, tag="wk")
                      sbf = epi.tile([P, out_f], F32, tag="sbf")
                      for k in range(3):
                          nc.vector.tensor_mul(scr4[:], score[:],
                                               cb_s[:, k * 4:(k + 1) * 4])
                          nc.vector.tensor_reduce(wk[:, k:k + 1], scr4[:],
                                                  AX.X, ALU.add)
                      for j in range(out_f):
                          nc.vector.tensor_mul(scr4[:], score[:],
                                               bfcb_s[:, j * 4:(j + 1) * 4])
                          nc.vector.tensor_reduce(sbf[:, j:j + 1], scr4[:],
                                                  AX.X, ALU.add)
                      tgl = (m0 + s0) // P
                      ob = outbuf[:, tgl * out_f:(tgl + 1) * out_f]
                      scr2 = epi.tile([P, out_f], F32, tag="scr2")
                      nc.vector.tensor_scalar_mul(ob, py[:, 0:out_f], wk[:, 0:1])
                      nc.vector.tensor_scalar_mul(scr2[:], py[:, out_f:2 * out_f],
                                                  wk[:, 1:2])
                      nc.vector.tensor_add(ob, ob, scr2[:])
                      nc.vector.tensor_scalar_mul(scr2[:], py[:, 2 * out_f:3 * out_f],
                                                  wk[:, 2:3])
                      nc.vector.tensor_add(ob, ob, scr2[:])
                      nc.vector.tensor_add(ob, ob, sbf[:])

            nc.sync.dma_start(
                out=out_d[:].rearrange("(t p) j -> p t j", p=P),
                in_=outbuf[:].rearrange("p (t j) -> p t j", j=out_f))
    nc.compile()
    return nc


def numpy_model(cfg, in_maps, G_total, plan):
    """Bit-approximate numpy model of what the device computes (f32 math),
    for validating the kernel structure without hardware."""
    ncore, nloc_pad, hid, out_f = cfg.ncore, cfg.nloc_pad, cfg.hid, cfg.out_f
    B = G_total * P
    outs = []
    h_tabs = []
    for c in range(ncore):
        im = in_maps[c]
        xT = im["xT"].astype(np.float32)
        w1 = im["w1"].astype(np.float32)
        w2 = im["w2"].astype(np.float32)
        t1 = np.maximum(xT.T @ w1, 0.0)
        t1 = np.concatenate([t1, np.ones((nloc_pad, 1), np.float32)], 1)
        h = t1 @ w2
        h_tabs.append(h.astype(np.float16).astype(np.float32))

    R = nloc_pad // NSLC

    def prop_all(tabs):
        full = np.concatenate(
            [tabs[c][s * R:(s + 1) * R] for s in range(NSLC)
             for c in range(ncore)], 0)
        res = []
        for c in range(ncore):
            im = in_maps[c]
            out = np.zeros((nloc_pad, hid), np.float32)
            gi_f = im["gi"][:16, :].T.reshape(-1)
            do_f = im["do"].T.reshape(-1)
            ew_f = im["ew"].T.reshape(-1)
            off = 0
            for (b, q, g, _nv) in plan:
                ch = g * P
                gidx = gi_f[off:off + ch].astype(np.int64)
                dof = do_f[off:off + ch]
                ewf = ew_f[off:off + ch]
                m = gidx >= 0
                rows = np.zeros((ch, hid), np.float32)
                rows[m] = full[q * QROWS + gidx[m], :].astype(
                    np.float16).astype(np.float32)
                dloc = dof.astype(np.int64)
                valid = (dloc >= 0) & (dloc < P)
                acc = np.zeros((P, hid), np.float32)
                np.add.at(acc, dloc[valid],
                          (ewf[valid, None].astype(np.float16).astype(np.float32)
                           * rows[valid]))
                out[b * P:(b + 1) * P] += acc
                off += ch
            res.append(out.astype(np.float16).astype(np.float32))
        return res

    p1_tabs = prop_all(h_tabs)
    p2_tabs = prop_all(p1_tabs)

    for c in range(ncore):
        im = in_maps[c]
        hT = h_tabs[c]
        p1 = p1_tabs[c]
        p2 = p2_tabs[c]
        stack = np.concatenate([hT, p1, p2, np.ones((nloc_pad, 1), np.float32)], 1)
        comb = stack @ im["wcomb"].astype(np.float32)
        hproj = np.tanh(comb).reshape(nloc_pad, 4, hid)
        xp = np.tanh(np.concatenate([hT, np.ones((nloc_pad, 1), np.float32)], 1)
                     @ im["wxp"].astype(np.float32))
        logits = np.einsum("nfd,nd->nf", hproj, xp)
        e = np.exp(logits - logits.max(1, keepdims=True))
        score = e / e.sum(1, keepdims=True)
        y = (stack[:, :3 * hid] @ im["wy"].astype(np.float32)).reshape(
            nloc_pad, 3, out_f)
        wk = np.stack([(score * im["cb"][0, k * 4:(k + 1) * 4][None, :]).sum(1)
                       for k in range(3)], 1)
        sbf = np.stack([(score * im["bfcb"][0, j * 4:(j + 1) * 4][None, :]).sum(1)
                        for j in range(out_f)], 1)
        out = (y * wk[:, :, None]).sum(1) + sbf
        outs.append(out.astype(np.float32))
    return outs


# ---------------------------------------------------------------------------
# Self-contained harness entry point: kernel(**inputs) -> np.ndarray
# ---------------------------------------------------------------------------
_NC_CACHE = {}


def kernel(**inputs):
    """AMNet forward on 8 TRN2 NeuronCores. Takes full unsharded inputs,
    returns the full [N, 2] float32 output."""
    from concourse.bass_utils import run_bass_kernel_spmd

    cfg = Cfg(n=100000, ncore=8, in_f=166, hid=156, out_f=2)
    in_maps, G_total, plan = host_preprocess(cfg, **inputs)
    key = (G_total, tuple(plan))
    nc = _NC_CACHE.get(key)
    if nc is None:
        nc = build(cfg, G_total, plan)
        _NC_CACHE[key] = nc
    res = run_bass_kernel_spmd(nc, in_maps,
                               core_ids=list(range(cfg.ncore)), trace=False)
    out = np.concatenate(
        [res.results[i]["out"][:cfg.nloc] for i in range(cfg.ncore)], 0)
    return out.astype(np.float32)


# revision 29
# speedup vs baseline: 2.4872x; 1.1000x over previous
"""AMNet (BernNet-style GNN) distributed Bass kernel for 8 TRN2 NeuronCores.

Math reformulation (K=2 Bernstein basis):
  reference does 5 sparse props; but with p0 = h, p1 = A_hat h, p2 = A_hat p1:
    B0 = (p0 + 2 p1 + p2)/4,  B1 = (p0 - p2)/2,  B2 = (p0 - 2 p1 + p2)/4
  so only TWO sparse propagations are needed.
  filters: filt_f = sum_k c[f,k] p_k + b_filt[f],  c = relu(theta) @ M
  attention epilogue fully refactored into matmuls (see build()).

Distribution: nodes sharded over 8 cores (12500 each). Edges partitioned by
dst core. Each prop: AllGather the (padded fp16) node table to h_full, then
per 128-dst block: dma_gather the source rows (sorted by dst block), build a
weighted one-hot selector on DVE (sel[e,d] = ew[e] * (dstoff[e]==d)) and
segment-reduce on TensorE into a PSUM tile — NO dma_scatter_add, no
accumulator tables, no merge phase. Block results stream out sequentially.
"""

import math

import numpy as np

import concourse.bass as bass
import concourse.tile as tile
from concourse import bacc, library_config, mybir

FP16 = mybir.dt.float16
F32 = mybir.dt.float32
I16 = mybir.dt.int16
P = 128
AX = mybir.AxisListType
ALU = mybir.AluOpType
ACT = mybir.ActivationFunctionType

QROWS = 32768          # gather source slice rows (int16 index range)
NSLC = 7               # AllGather slices (nloc_pad must divide evenly)


def _patch_swdge_lane_assignment():
    """Tile round-robins DMASW sem lanes ignoring queue_num, but each lane is
    locked to one SWDGE queue by the ucode/sim. Pin lane = queue_num + 4*flip
    so multi-queue swdge DMAs get consistent lanes (8 lanes / 4 queues)."""
    import concourse.tile_sem_assignment as tsa
    if getattr(tsa, "_amnet_lane_patch", False):
        return
    tsa._amnet_lane_patch = True
    orig = tsa.TileClockTick._assign_tick

    def _assign_tick(self, inst):
        if (isinstance(inst, tsa.DMAInst)
                and inst.engine == tsa.mybir.EngineType.Pool
                and not isinstance(inst, tsa.bass_isa.UserSyncedRemoteDMADescs)):
            q = getattr(inst, "queue_num", 0) or 0
            flips = getattr(self, "_amnet_qflip", None)
            if flips is None:
                flips = self._amnet_qflip = [0, 0, 0, 0]
            lane = q + 4 * flips[q]
            flips[q] ^= 1
            save = self.next_sw_dma_idx
            self.next_sw_dma_idx = lane
            try:
                return orig(self, inst)
            finally:
                self.next_sw_dma_idx = save
        return orig(self, inst)

    tsa.TileClockTick._assign_tick = _assign_tick


_patch_swdge_lane_assignment()


class Cfg:
    def __init__(self, n, ncore, in_f, hid, out_f):
        assert n % ncore == 0
        self.n = n
        self.ncore = ncore
        self.nloc = n // ncore
        self.in_f = in_f
        self.hid = hid
        self.out_f = out_f
        self.nloc_pad = ((self.nloc + P - 1) // P) * P
        self.tiles = self.nloc_pad // P       # dst blocks per core
        self.padw = 256            # fp16 table row elems (512B, 256B-multiple)
        self.gcap = 8              # dma_gather num_idxs cap is 1024 (HW)
        self.gbufs = 8             # gather buffer ring size
        # MLP node-chunk size (PSUM free limit 512)
        self.mlp_chunk = 512


def _wrap16(a, pad_val, total):
    """idx array -> [128, total//16] int16 in the dma_gather wrapped layout."""
    out = np.full(total, pad_val, dtype=np.int16)
    out[: a.shape[0]] = a.astype(np.int16)
    w = out.reshape(total // 16, 16).T  # elem j -> [j%16, j//16]
    return np.tile(w, (8, 1)).copy()   # replicated for the 8 gpsimd cores


def host_preprocess(cfg, x, edge_index, W1, b1, W2, b2, theta, b_filt,
                    Wf, bf, Wx, bx, Wc, bc):
    """Build per-core input maps. Returns (in_maps, G_total, plan)."""
    n, ncore, nloc = cfg.n, cfg.ncore, cfg.nloc
    nloc_pad, hid, in_f, out_f = cfg.nloc_pad, cfg.hid, cfg.in_f, cfg.out_f

    src = np.asarray(edge_index[0], dtype=np.int64)
    dst = np.asarray(edge_index[1], dtype=np.int64)
    deg = np.bincount(dst, minlength=n).astype(np.float32)
    dinv = (1.0 / np.sqrt(np.maximum(deg, 1.0))).astype(np.float32)
    ewv = dinv[src] * dinv[dst]

    # global row in the slice-major full table: the AllGather is emitted in
    # NSLC row-slices, so the full table is laid out [slice][core][row]
    R = nloc_pad // NSLC
    sc, sr = src // nloc, src % nloc
    src_row = (sr // R) * (ncore * R) + sc * R + (sr % R)
    nq = (ncore * nloc_pad + QROWS - 1) // QROWS

    ecore = dst // nloc

    # Per core: sort edges by (dst block, src slice q, src_row).
    per_core = []
    cnts = np.zeros((ncore, cfg.tiles, nq), np.int64)
    for c in range(ncore):
        sel = ecore == c
        es_row, ed, ev = src_row[sel], dst[sel] - c * nloc, ewv[sel]
        blk = ed // P
        q = es_row // QROWS
        o = np.lexsort((es_row, q, blk))
        es_row, ed, ev, blk, q = es_row[o], ed[o], ev[o], blk[o], q[o]
        per_core.append((es_row, ed, ev))
        np.add.at(cnts[c], (blk, q), 1)

    # plan: [(block, q, ngroups, nvalid)] identical across cores; per-entry
    # slot counts are maxed over cores and rounded up to whole 128-groups.
    # nvalid = number of non-skipped descriptors in the call (the gather
    # ucode requires num_idxs_reg == count of non-negative indices, so every
    # core pads its real edges with idx=0 descriptors up to nvalid).
    maxc = cnts.max(axis=0)  # [tiles, nq]
    plan = []
    for b in range(cfg.tiles):
        for q in range(nq):
            mc = int(maxc[b, q])
            g = (mc + P - 1) // P
            off = 0
            while g > 0:
                take = min(g, cfg.gcap)
                nvalid = min(mc - off, take * P)
                plan.append((b, q, take, nvalid))
                off += take * P
                g -= take
    G_total = sum(g for (_, _, g, _) in plan)
    B = G_total * P

    # ---- weights ----
    h2 = hid - P
    thr = np.maximum(np.asarray(theta, np.float64), 0.0)           # relu
    M = np.array([[.25, .5, .25], [.5, 0., -.5], [.25, -.5, .25]], np.float64)
    c3 = (thr @ M)                                                 # [4,3]

    W1 = np.asarray(W1, np.float64); W2 = np.asarray(W2, np.float64)
    Wf = np.asarray(Wf, np.float64); Wx = np.asarray(Wx, np.float64)
    Wc = np.asarray(Wc, np.float64)
    b1 = np.asarray(b1, np.float64); b2 = np.asarray(b2, np.float64)
    bf = np.asarray(bf, np.float64); bx = np.asarray(bx, np.float64)
    bc = np.asarray(bc, np.float64); bflt = np.asarray(b_filt, np.float64)

    w1p = np.concatenate([W1, b1[None, :]], 0).astype(np.float16)      # [in_f+1, hid]
    w2p = np.concatenate([W2, b2[None, :]], 0).astype(np.float16)      # [hid+1, hid]

    wcomb = np.zeros((3 * hid + 1, 4 * hid), np.float64)
    for f in range(4):
        for k in range(3):
            wcomb[k * hid:(k + 1) * hid, f * hid:(f + 1) * hid] = c3[f, k] * Wf
        wcomb[3 * hid, f * hid:(f + 1) * hid] = bflt[f] @ Wf + bf

    # epilogue rhs blocks: psum A = [comb_f01 (312) | xpre (156)],
    # psum B = [comb_f23 (312) | y j-major (6)] — per-k stationary rhs.
    rca = np.zeros((3, hid, 2 * hid + hid), np.float64)  # [k, hid, 468]
    rcb = np.zeros((3, hid, 2 * hid + 3 * out_f), np.float64)
    for k in range(3):
        rca[k, :, 0:2 * hid] = wcomb[k * hid:(k + 1) * hid, 0:2 * hid]
        rcb[k, :, 0:2 * hid] = wcomb[k * hid:(k + 1) * hid, 2 * hid:4 * hid]
        for j in range(out_f):
            rcb[k, :, 2 * hid + j * 3 + k] = Wc[:, j]
    rca[0, :, 2 * hid:] = Wx
    rca = rca.astype(np.float16)
    rcb = rcb.astype(np.float16)
    biasA = np.zeros((1, 2 * hid + hid), np.float64)
    biasA[0, 0:2 * hid] = wcomb[3 * hid, 0:2 * hid]
    biasA[0, 2 * hid:] = bx
    biasB = np.zeros((1, 2 * hid + 3 * out_f), np.float64)
    biasB[0, 0:2 * hid] = wcomb[3 * hid, 2 * hid:4 * hid]
    biasA = biasA.astype(np.float16)
    biasB = biasB.astype(np.float16)

    # combined score-weight table: cols g*4+f, groups 0..2 -> c3[f,k],
    # groups 3..4 -> bflt@Wc + bc
    bfc = bflt @ Wc                              # [4, out_f]
    cbf = np.zeros((P, 20), np.float32)
    for k in range(3):
        for f in range(4):
            cbf[:, k * 4 + f] = c3[f, k]
    for j in range(out_f):
        for f in range(4):
            cbf[:, (3 + j) * 4 + f] = bfc[f, j] + bc[j]

    tidx = _wrap16(np.arange(cfg.mlp_chunk), 0, cfg.mlp_chunk)

    x = np.asarray(x, np.float32)

    in_maps = []
    for c in range(ncore):
        xT = np.zeros((in_f + 1, nloc_pad), np.float16)
        xT[:in_f, :nloc] = x[c * nloc:(c + 1) * nloc].T
        xT[in_f, :] = 1.0

        es_row, ed, ev = per_core[c]
        blk = ed // P
        q = es_row // QROWS
        # per (b, q) run boundaries in the sorted arrays
        # fill flat slot arrays per plan entry
        gflat = np.full(B, -1, np.int64)       # -1 = skipped descriptor
        doflat = np.full(B, -1.0, np.float32)  # -1 = sel matches nothing
        ewflat = np.zeros(B, np.float32)
        key = blk * nq + q
        starts = np.searchsorted(key, np.arange(cfg.tiles * nq), side="left")
        ends = np.searchsorted(key, np.arange(cfg.tiles * nq), side="right")
        consumed = {}
        off = 0
        for (b, qq, g, nvalid) in plan:
            kidx = b * nq + qq
            s0, s1 = int(starts[kidx]), int(ends[kidx])
            done = consumed.get(kidx, 0)
            take = max(0, min(g * P, (s1 - s0) - done))
            if take > 0:
                sl = slice(s0 + done, s0 + done + take)
                gflat[off:off + take] = es_row[sl] - qq * QROWS
                doflat[off:off + take] = (ed[sl] - b * P).astype(np.float32)
                ewflat[off:off + take] = ev[sl]
            # pad with idx=0 descriptors (sel-zeroed) up to the call's
            # shared valid count; the rest stay -1 (skipped)
            if nvalid > take:
                gflat[off + take:off + nvalid] = 0
            consumed[kidx] = done + take
            off += g * P
        assert off == B

        gi = np.tile(gflat.astype(np.int16).reshape(B // 16, 16).T,
                     (8, 1)).copy()
        # slot j of group g -> partition j%128; DVE tables are
        # [P, G_total] with column g holding slots [g*128, (g+1)*128)
        dot = doflat.reshape(G_total, P).T.astype(np.float16).copy()
        ewt = ewflat.reshape(G_total, P).T.astype(np.float16).copy()

        in_maps.append({
            "xT": xT,
            "w1": w1p, "w2": w2p,
            "rca": rca, "rcb": rcb, "biasA": biasA, "biasB": biasB,
            "cbf": cbf,
            "gi": gi, "do": dot, "ew": ewt, "tidx": tidx,
        })
    return in_maps, G_total, plan


def build(cfg, G_total, plan):
    """Build the SPMD Bass graph. All cores run this same program."""
    ncore, nloc_pad, hid, in_f, out_f, padw = (
        cfg.ncore, cfg.nloc_pad, cfg.hid, cfg.in_f, cfg.out_f, cfg.padw)
    B = G_total * P
    nperm = B // 16
    rg = [list(range(ncore))]
    h2 = hid - P            # 28
    MC = cfg.mlp_chunk
    nq = (ncore * nloc_pad + QROWS - 1) // QROWS

    nc = bacc.Bacc(None, num_devices=ncore, num_swdge_queues=4)

    FA = 2 * hid + hid            # psum A cols: comb_f01 | xpre
    FB = 2 * hid + 3 * out_f      # psum B cols: comb_f23 | y (j-major)

    dp = nc.declare_dram_parameter
    xT_d = dp("xT", [in_f + 1, nloc_pad], FP16, isOutput=False)
    w1_d = dp("w1", [in_f + 1, hid], FP16, isOutput=False)
    w2_d = dp("w2", [hid + 1, hid], FP16, isOutput=False)
    rca_d = dp("rca", [3, hid, FA], FP16, isOutput=False)
    rcb_d = dp("rcb", [3, hid, FB], FP16, isOutput=False)
    biasA_d = dp("biasA", [1, FA], FP16, isOutput=False)
    biasB_d = dp("biasB", [1, FB], FP16, isOutput=False)
    cbf_d = dp("cbf", [P, 20], F32, isOutput=False)
    gi_d = dp("gi", [P, nperm], I16, isOutput=False)
    do_d = dp("do", [P, G_total], FP16, isOutput=False)
    ew_d = dp("ew", [P, G_total], FP16, isOutput=False)
    tidx_d = dp("tidx", [P, MC // 16], I16, isOutput=False)
    out_d = dp("out", [nloc_pad, out_f], F32, isOutput=True)

    h_tab = nc.dram_tensor("h_tab", [nloc_pad, padw], FP16)
    p1_tab = nc.dram_tensor("p1_tab", [nloc_pad, padw], FP16)
    p2_tab = nc.dram_tensor("p2_tab", [nloc_pad, padw], FP16)
    h_full = nc.dram_tensor("h_full", [ncore * nloc_pad, padw], FP16,
                            addr_space="Shared")
    p1_full = nc.dram_tensor("p1_full", [ncore * nloc_pad, padw], FP16,
                             addr_space="Shared")

    with tile.TileContext(nc, num_cores=ncore) as tc:
        with tc.tile_pool(name="res", bufs=1) as res:
            # ---------------- resident loads ----------------
            def load(pool, dram, shape, dt, name):
                t = pool.tile(shape, dt, name=name, tag=name)
                nc.sync.dma_start(out=t[:], in_=dram[:])
                return t

            w1a = load(res, w1_d[0:P, :], [P, hid], FP16, "w1a")
            w1b = load(res, w1_d[P:in_f + 1, :], [in_f + 1 - P, hid], FP16, "w1b")
            w2a = load(res, w2_d[0:P, :], [P, hid], FP16, "w2a")
            w2b = load(res, w2_d[P:hid + 1, :], [hid + 1 - P, hid], FP16, "w2b")
            gi_s = load(res, gi_d[:, :], [P, nperm], I16, "gi_s")
            do_s = load(res, do_d[:, :], [P, G_total], FP16, "do_s")
            ew_s = load(res, ew_d[:, :], [P, G_total], FP16, "ew_s")
            tidx_s = load(res, tidx_d[:, :], [P, MC // 16], I16, "tidx_s")
            rcaa = [load(res, rca_d[k, 0:P, :], [P, FA], FP16, f"rcaa{k}")
                    for k in range(3)]
            rcab = [load(res, rca_d[k, P:hid, :], [h2, FA], FP16, f"rcab{k}")
                    for k in range(3)]
            rcba = [load(res, rcb_d[k, 0:P, :], [P, FB], FP16, f"rcba{k}")
                    for k in range(3)]
            rcbb = [load(res, rcb_d[k, P:hid, :], [h2, FB], FP16, f"rcbb{k}")
                    for k in range(3)]
            biasA_s = load(res, biasA_d, [1, FA], FP16, "biasA_s")
            biasB_s = load(res, biasB_d, [1, FB], FP16, "biasB_s")
            cbf_s = load(res, cbf_d, [P, 20], F32, "cbf_s")

            ones1 = res.tile([1, P], FP16)
            nc.vector.memset(ones1[:], 1.0)
            outbuf = res.tile([P, cfg.tiles * out_f], F32)

            # iota row 0..127 on every partition (exact in fp16)
            iota_s = res.tile([P, P], FP16, name="iota_s")
            nc.gpsimd.iota(iota_s[:], pattern=[[1, P]], base=0,
                           channel_multiplier=0,
                           allow_small_or_imprecise_dtypes=True)

            # gather buffer ring — memset once so slots skipped by negative
            # indices stay finite (sel multiplies them by 0)
            g_bufs = [res.tile([P, cfg.gcap, padw], FP16, name=f"gbuf{i}")
                      for i in range(cfg.gbufs)]
            for gb in g_bufs:
                nc.vector.memset(gb[:], 0.0)

            # sliced AllGather: slice s covers local rows [s*R, (s+1)*R) and
            # lands contiguously at full[(s*ncore + core)*R, ...]
            R = nloc_pad // NSLC

            def ag_slice(src_tab, dst_full, s):
                nc.gpsimd.collective_compute(
                    "AllGather", ALU.bypass, replica_groups=rg,
                    ins=[src_tab[s * R:(s + 1) * R, :]],
                    outs=[dst_full[s * ncore * R:(s + 1) * ncore * R, :]])

            # ---------------- MLP: h = relu(x@W1+b1)@W2+b2 ----------------
            with (
                tc.tile_pool(name="mlp", bufs=3) as mlp,
                tc.tile_pool(name="psm", bufs=2, space="PSUM") as psA,
                tc.tile_pool(name="psm2", bufs=2, space="PSUM") as psB,
            ):
              ag_done = 0
              for m0 in range(0, nloc_pad, MC):
                  F = min(MC, nloc_pad - m0)
                  xa = mlp.tile([P, F], FP16, tag="xa")
                  xb = mlp.tile([in_f + 1 - P, F], FP16, tag="xb")
                  nc.sync.dma_start(out=xa[:], in_=xT_d[0:P, m0:m0 + F])
                  nc.sync.dma_start(out=xb[:], in_=xT_d[P:in_f + 1, m0:m0 + F])
                  pa = psA.tile([P, F], F32, tag="mlp_pa")
                  pb = psA.tile([h2, F], F32, tag="mlp_pb")
                  nc.tensor.matmul(pa[:], w1a[:, 0:P], xa[:], start=True, stop=False)
                  nc.tensor.matmul(pa[:], w1b[:, 0:P], xb[:], start=False, stop=True)
                  nc.tensor.matmul(pb[:], w1a[:, P:hid], xa[:], start=True, stop=False)
                  nc.tensor.matmul(pb[:], w1b[:, P:hid], xb[:], start=False, stop=True)
                  t1a = mlp.tile([P, F], FP16, tag="t1a")
                  t1b = mlp.tile([hid + 1 - P, F], FP16, tag="t1b")
                  nc.scalar.activation(t1a[:], pa[:], ACT.Relu)
                  nc.vector.memset(t1b[:], 1.0)
                  nc.scalar.activation(t1b[0:h2, :], pb[:], ACT.Relu)
                  for s0 in range(0, F, P):
                      hp = psB.tile([P, hid], F32, tag="mlp_hp")
                      nc.tensor.matmul(hp[:], t1a[:, s0:s0 + P], w2a[:],
                                       start=True, stop=False)
                      nc.tensor.matmul(hp[:], t1b[:, s0:s0 + P], w2b[:],
                                       start=False, stop=True)
                      hrow = mlp.tile([P, hid], FP16, tag="hrow")
                      nc.scalar.activation(hrow[:], hp[:], ACT.Copy)
                      nc.sync.dma_start(
                          out=h_tab[m0 + s0:m0 + s0 + P, 0:hid], in_=hrow[:])
                  while (ag_done + 1) * R <= m0 + F:
                      ag_slice(h_tab, h_full, ag_done)
                      ag_done += 1
              assert ag_done == NSLC

            # ---------------- propagation (segment-sum via matmul) --------
            # plan entries grouped per block
            blk_entries = {}
            eoff = 0
            for (b, q, g, nvalid) in plan:
                blk_entries.setdefault(b, []).append((q, g, nvalid, eoff))
                eoff += g

            gctr = [0]
            maxg = max(sum(g for (_, g, _, _) in es)
                       for es in blk_entries.values())

            def prop(gat, psp, src_full, dst_tab, ag_dst=None, blk_cb=None):
                qsl = [src_full[q * QROWS:min((q + 1) * QROWS,
                                              ncore * nloc_pad), :]
                       for q in range(nq)]
                qn = 0
                blk_per_slice = (nloc_pad // P) // NSLC
                for b in range(cfg.tiles):
                    entries = blk_entries.get(b, [])
                    nmm = sum(g for (_, g, _, _) in entries)
                    # two PSUM accumulators per block (even/odd groups) so
                    # consecutive matmuls alternate banks
                    nps = 2 if nmm > 1 else 1
                    ps = [psp.tile([P, hid], F32, tag=f"segp{i}",
                                   name=f"segp{i}")
                          for i in range(nps)]
                    started = [False] * nps
                    g0 = entries[0][3] if entries else 0
                    # batched selector build: sel[p, k, d] =
                    #   ew[p, g0+k] * (iota[d] == dstoff[p, g0+k])
                    sel = gat.tile([P, nmm, P], FP16, tag="sel",
                                   padded_shape=[P, maxg, P])
                    nc.vector.tensor_tensor(
                        out=sel[:],
                        in0=iota_s[:].unsqueeze(1).broadcast_to((P, nmm, P)),
                        in1=do_s[:, g0:g0 + nmm].to_broadcast((P, nmm, P)),
                        op=ALU.is_equal)
                    nc.vector.tensor_tensor(
                        out=sel[:], in0=sel[:],
                        in1=ew_s[:, g0:g0 + nmm].to_broadcast((P, nmm, P)),
                        op=ALU.mult)
                    mi = 0
                    for (q, g, nvalid, goff) in entries:
                        gb = g_bufs[gctr[0] % cfg.gbufs]
                        gctr[0] += 1
                        i16 = goff * 8  # = goff*128/16
                        nc.gpsimd.dma_gather(
                            gb[:, 0:g, :], qsl[q],
                            gi_s[:, i16:i16 + g * 8],
                            g * P, nvalid, padw, queue_num=qn % 4)
                        qn += 1
                        for k in range(g):
                            pi = mi % nps
                            nc.tensor.matmul(ps[pi][:], sel[:, goff - g0 + k, :],
                                             gb[:, k, 0:hid],
                                             start=(not started[pi]),
                                             stop=(mi >= nmm - nps))
                            started[pi] = True
                            mi += 1
                    ob = gat.tile([P, hid], FP16, tag="ob")
                    if nmm == 0:
                        nc.vector.memset(ob[:], 0.0)
                    elif nps == 1:
                        nc.scalar.activation(ob[:], ps[0][:], ACT.Copy)
                    else:
                        nc.scalar.activation(ob[:], ps[0][:], ACT.Copy)
                        nc.vector.tensor_add(ob[:], ob[:], ps[1][:])
                    nc.sync.dma_start(
                        out=dst_tab[b * P:(b + 1) * P, 0:hid], in_=ob[:])
                    if ag_dst is not None and (b + 1) % blk_per_slice == 0:
                        ag_slice(dst_tab, ag_dst, (b + 1) // blk_per_slice - 1)
                    if blk_cb is not None:
                        blk_cb(b)

            with (
                tc.tile_pool(name="gat", bufs=6) as gat,
                tc.tile_pool(name="psp", bufs=2, space="PSUM") as psp,
                tc.tile_pool(name="epi", bufs=2) as epi,
                tc.tile_pool(name="pse", bufs=2, space="PSUM") as psE,
            ):
                def epi_chunk(m0):
                    """Attention epilogue for node rows [m0, m0+MC)."""
                    F = min(MC, nloc_pad - m0)
                    tT = []
                    for ti, tab in enumerate((h_tab, p1_tab, p2_tab)):
                        t = epi.tile([P, padw // P, F], FP16, tag=f"tT{ti}",
                                     name=f"tT{ti}")
                        nc.gpsimd.dma_gather(
                            t[:], tab[m0:m0 + F, :],
                            tidx_s[:, 0:F // 16], F, F, padw, transpose=True,
                            queue_num=ti % 4)
                        tT.append(t)
                    for s0 in range(0, F, P):
                        sl = slice(s0, s0 + P)
                        pcA = psE.tile([P, FA], F32, tag="pcA")
                        pcB = psE.tile([P, FB], F32, tag="pcB")
                        for k in range(3):
                            nc.tensor.matmul(pcA[:], tT[k][:, 0, sl], rcaa[k][:],
                                             start=(k == 0), stop=False)
                            nc.tensor.matmul(pcB[:], tT[k][:, 0, sl], rcba[k][:],
                                             start=(k == 0), stop=False)
                            nc.tensor.matmul(pcA[:], tT[k][0:h2, 1, sl],
                                             rcab[k][:], start=False, stop=False)
                            nc.tensor.matmul(pcB[:], tT[k][0:h2, 1, sl],
                                             rcbb[k][:], start=False, stop=False)
                        nc.tensor.matmul(pcA[:], ones1[:], biasA_s[:],
                                         start=False, stop=True)
                        nc.tensor.matmul(pcB[:], ones1[:], biasB_s[:],
                                         start=False, stop=True)
                        hp = epi.tile([P, 4, hid], FP16, tag="hp")
                        xp = epi.tile([P, hid], FP16, tag="xp")
                        nc.scalar.activation(hp[:, 0:2, :], pcA[:, 0:2 * hid],
                                             ACT.Tanh)
                        nc.scalar.activation(hp[:, 2:4, :], pcB[:, 0:2 * hid],
                                             ACT.Tanh)
                        nc.scalar.activation(xp[:], pcA[:, 2 * hid:FA], ACT.Tanh)
                        prod = epi.tile([P, 4, hid], FP16, tag="prod")
                        nc.vector.tensor_tensor(
                            out=prod[:], in0=hp[:],
                            in1=xp[:].unsqueeze(1).broadcast_to((P, 4, hid)),
                            op=ALU.mult)
                        logit = epi.tile([P, 4], F32, tag="logit")
                        nc.vector.tensor_reduce(logit[:], prod[:], AX.X, ALU.add)
                        mxn = epi.tile([P, 1], F32, tag="mxn")
                        nc.vector.tensor_reduce(mxn[:], logit[:], AX.X, ALU.max,
                                                negate=True)
                        el = epi.tile([P, 4], F32, tag="el")
                        nc.scalar.activation(el[:], logit[:], ACT.Exp,
                                             bias=mxn[:, 0:1])
                        sm = epi.tile([P, 1], F32, tag="sm")
                        nc.vector.tensor_reduce(sm[:], el[:], AX.X, ALU.add)
                        rs = epi.tile([P, 1], F32, tag="rs")
                        nc.vector.reciprocal(rs[:], sm[:])
                        score = epi.tile([P, 4], F32, tag="score")
                        nc.vector.tensor_scalar_mul(score[:], el[:], rs[:, 0:1])
                        prod2 = epi.tile([P, 5, 4], F32, tag="prod2")
                        nc.vector.tensor_tensor(
                            out=prod2[:],
                            in0=cbf_s[:].rearrange("p (g f) -> p g f", f=4),
                            in1=score[:].unsqueeze(1).broadcast_to((P, 5, 4)),
                            op=ALU.mult)
                        wsb = epi.tile([P, 5], F32, tag="wsb")
                        nc.vector.tensor_reduce(wsb[:], prod2[:], AX.X, ALU.add)
                        prod3 = epi.tile([P, out_f, 3], F32, tag="prod3")
                        nc.vector.tensor_tensor(
                            out=prod3[:],
                            in0=pcB[:, 2 * hid:FB].rearrange(
                                "p (j k) -> p j k", k=3),
                            in1=wsb[:, 0:3].unsqueeze(1).broadcast_to(
                                (P, out_f, 3)),
                            op=ALU.mult)
                        tgl = (m0 + s0) // P
                        ob2 = outbuf[:, tgl * out_f:(tgl + 1) * out_f]
                        nc.vector.tensor_reduce(ob2, prod3[:], AX.X, ALU.add)
                        nc.vector.tensor_add(ob2, ob2, wsb[:, 3:5])

                prop(gat, psp, h_full, p1_tab, ag_dst=p1_full)

                # prop2 with the epilogue fused in: chunk m covers blocks
                # [4m, 4m+4); emit it once block 4(m+1) is done (1-block lag)
                emitted = [0]
                nchunks = (nloc_pad + MC - 1) // MC

                def blk_cb(b):
                    while (emitted[0] + 1) * (MC // P) + 1 <= b + 1:
                        epi_chunk(emitted[0] * MC)
                        emitted[0] += 1

                prop(gat, psp, p1_full, p2_tab, blk_cb=blk_cb)
                while emitted[0] < nchunks:
                    epi_chunk(emitted[0] * MC)
                    emitted[0] += 1

            nc.sync.dma_start(
                out=out_d[:].rearrange("(t p) j -> p t j", p=P),
                in_=outbuf[:].rearrange("p (t j) -> p t j", j=out_f))
    nc.compile()
    return nc


def numpy_model(cfg, in_maps, G_total, plan):
    """Bit-approximate numpy model of what the device computes (f32 math),
    for validating the kernel structure without hardware."""
    ncore, nloc_pad, hid, out_f = cfg.ncore, cfg.nloc_pad, cfg.hid, cfg.out_f
    B = G_total * P
    outs = []
    h_tabs = []
    for c in range(ncore):
        im = in_maps[c]
        xT = im["xT"].astype(np.float32)
        w1 = im["w1"].astype(np.float32)
        w2 = im["w2"].astype(np.float32)
        t1 = np.maximum(xT.T @ w1, 0.0)
        t1 = np.concatenate([t1, np.ones((nloc_pad, 1), np.float32)], 1)
        h = t1 @ w2
        h_tabs.append(h.astype(np.float16).astype(np.float32))

    R = nloc_pad // NSLC

    def prop_all(tabs):
        full = np.concatenate(
            [tabs[c][s * R:(s + 1) * R] for s in range(NSLC)
             for c in range(ncore)], 0)
        res = []
        for c in range(ncore):
            im = in_maps[c]
            out = np.zeros((nloc_pad, hid), np.float32)
            gi_f = im["gi"][:16, :].T.reshape(-1)
            do_f = im["do"].T.reshape(-1)
            ew_f = im["ew"].T.reshape(-1)
            off = 0
            for (b, q, g, _nv) in plan:
                ch = g * P
                gidx = gi_f[off:off + ch].astype(np.int64)
                dof = do_f[off:off + ch]
                ewf = ew_f[off:off + ch]
                m = gidx >= 0
                rows = np.zeros((ch, hid), np.float32)
                rows[m] = full[q * QROWS + gidx[m], :].astype(
                    np.float16).astype(np.float32)
                dloc = dof.astype(np.int64)
                valid = (dloc >= 0) & (dloc < P)
                acc = np.zeros((P, hid), np.float32)
                np.add.at(acc, dloc[valid],
                          (ewf[valid, None].astype(np.float16).astype(np.float32)
                           * rows[valid]))
                out[b * P:(b + 1) * P] += acc
                off += ch
            res.append(out.astype(np.float16).astype(np.float32))
        return res

    p1_tabs = prop_all(h_tabs)
    p2_tabs = prop_all(p1_tabs)

    for c in range(ncore):
        im = in_maps[c]
        tabs = [h_tabs[c], p1_tabs[c], p2_tabs[c]]
        rca = im["rca"].astype(np.float32)
        rcb = im["rcb"].astype(np.float32)
        pcA = sum(tabs[k] @ rca[k] for k in range(3)) + im["biasA"].astype(
            np.float32)
        pcB = sum(tabs[k] @ rcb[k] for k in range(3)) + im["biasB"].astype(
            np.float32)
        hproj = np.tanh(np.concatenate(
            [pcA[:, :2 * hid], pcB[:, :2 * hid]], 1)).reshape(nloc_pad, 4, hid)
        xp = np.tanh(pcA[:, 2 * hid:])
        logits = np.einsum("nfd,nd->nf", hproj, xp)
        e = np.exp(logits - logits.max(1, keepdims=True))
        score = e / e.sum(1, keepdims=True)
        cbf = im["cbf"][0].reshape(5, 4)
        wsb = score @ cbf.T                       # [n, 5]
        py = pcB[:, 2 * hid:].reshape(nloc_pad, out_f, 3)
        out = (py * wsb[:, None, 0:3]).sum(2) + wsb[:, 3:5]
        outs.append(out.astype(np.float32))
    return outs


# ---------------------------------------------------------------------------
# Self-contained harness entry point: kernel(**inputs) -> np.ndarray
# ---------------------------------------------------------------------------
_NC_CACHE = {}


def kernel(**inputs):
    """AMNet forward on 8 TRN2 NeuronCores. Takes full unsharded inputs,
    returns the full [N, 2] float32 output."""
    from concourse.bass_utils import run_bass_kernel_spmd

    cfg = Cfg(n=100000, ncore=8, in_f=166, hid=156, out_f=2)
    in_maps, G_total, plan = host_preprocess(cfg, **inputs)
    key = (G_total, tuple(plan))
    nc = _NC_CACHE.get(key)
    if nc is None:
        nc = build(cfg, G_total, plan)
        _NC_CACHE[key] = nc
    res = run_bass_kernel_spmd(nc, in_maps,
                               core_ids=list(range(cfg.ncore)), trace=False)
    out = np.concatenate(
        [res.results[i]["out"][:cfg.nloc] for i in range(cfg.ncore)], 0)
    return out.astype(np.float32)
